# revision 13
# baseline (speedup 1.0000x reference)
"""Trainium2 Bass kernel for nn_NoFoDifformer_FourierKAN (8-core SPMD).

Sharding: u and nodes row-wise across 8 cores (1250 rows each). The [d,d]
K^T V Gram matrices and the chunked u^T h partial sums are all-reduced;
small weights are replicated; per-core row-shard outputs are concatenated
on the host.

The host pre-shards u into TWO bf16 tensors per core: u16 = u[rows,:] for
pass-1 (utx partials) and ut16 = u[rows,:].T for pass-2, both zero-padded
to 79*128 columns/rows. This removes the on-device transpose round-trip
entirely; the device streams each tensor once with large contiguous DMAs.
x is host-pre-transposed so the feature encoder needs no PE transposes of
x. LayerNorm affine params are folded into downstream projection weights.
"""

import math
from contextlib import ExitStack

import numpy as np

N_FULL = 10000
NF_FULL = 512
D = 128
CORES_FULL = 8
N_PAD = 10112                  # 79 * 128
LAMBDA_INIT = 0.2
CHUNK_LIST = [2048, 2048, 2048, 2048, 1920]        # psum chunks, sums to N_PAD
AR_GROUPS = [(0, 4096), (4096, 4096), (8192, 1920)]  # chunks per AllReduce


def _splits(total, step):
    return [(o, min(step, total - o)) for o in range(0, total, step)]


def build_kernel(N=N_FULL, NF=NF_FULL, CORES=CORES_FULL, sim_gelu=False):
    import concourse.bacc as bacc
    import concourse.tile as tile
    from concourse import mybir
    from concourse.masks import make_identity

    dt = mybir.dt
    f32 = dt.float32
    bf16 = dt.bfloat16
    AF = mybir.ActivationFunctionType
    ALU = mybir.AluOpType
    AX = mybir.AxisListType

    NLOC = N // CORES
    assert NLOC * CORES == N
    ROWS = _splits(NLOC, 128)          # per-core row tiles (i)
    NR = len(ROWS)
    KX = NF // 128                     # x feature k-tiles
    assert KX * 128 == NF
    NSUB = N_PAD // 128                # 79 j-subtiles
    chunks, off = [], 0
    for cw in CHUNK_LIST:
        chunks.append((off, cw))
        off += cw
    assert off == N_PAD
    NCH = len(chunks)
    IBLK = _splits(NLOC, 512)          # pass-2 output i blocks
    TG_FULL = N // 128                 # full 128-wide e subtiles
    TG_REM = N - TG_FULL * 128
    rg = [list(range(CORES))]
    shared_space = "Shared" if CORES > 4 else "Local"

    nc = bacc.Bacc("TRN2", target_bir_lowering=False, debug=False,
                   num_devices=CORES)

    # ---------------- DRAM I/O ----------------
    def din(name, shape):
        return nc.dram_tensor(name, list(shape), f32, kind="ExternalInput")

    t_xT = din("xT", (NF, NLOC))
    t_u16 = nc.dram_tensor("u16", [NLOC, N_PAD], bf16, kind="ExternalInput")
    t_ut16 = nc.dram_tensor("ut16", [N_PAD, NLOC], bf16, kind="ExternalInput")
    t_e = din("e", (N,))
    t_few1 = din("fe_w1", (NF, D)); t_feb1 = din("fe_b1", (D,))
    t_few2 = din("fe_w2", (D, D)); t_feb2 = din("fe_b2", (D,))
    t_kana = din("kan_a", (10,)); t_kanb = din("kan_b", (10,))
    t_kanbias = din("kan_bias", (1,)); t_alpha = din("alpha_w", (1, 1))
    t_mg = din("mha_ln_g", (D,)); t_mb = din("mha_ln_b", (D,))
    t_fg = din("ffn_ln_g", (D,)); t_fb = din("ffn_ln_b", (D,))
    t_q1w = din("q1_w", (D, D)); t_q1b = din("q1_b", (D,))
    t_k1w = din("k1_w", (D, D)); t_k1b = din("k1_b", (D,))
    t_q2w = din("q2_w", (D, D)); t_q2b = din("q2_b", (D,))
    t_k2w = din("k2_w", (D, D)); t_k2b = din("k2_b", (D,))
    t_vw = din("v_w", (D, D)); t_vb = din("v_b", (D,))
    t_ag = din("attn_ln_g", (D,)); t_ab = din("attn_ln_b", (D,))
    t_ow = din("out_w", (D, D)); t_ob = din("out_b", (D,))
    t_lq1 = din("lq1", (D,)); t_lk1 = din("lk1", (D,))
    t_lq2 = din("lq2", (D,)); t_lk2 = din("lk2", (D,))
    t_f1w = din("ffn1_w", (D, D)); t_f1b = din("ffn1_b", (D,))
    t_f2w = din("ffn2_w", (D, D)); t_f2b = din("ffn2_b", (D,))
    t_out = nc.dram_tensor("out", [NLOC, D], f32, kind="ExternalOutput")

    with tile.TileContext(nc) as tc, ExitStack() as ctx:
        wpool = ctx.enter_context(tc.tile_pool(name="wpool", bufs=1))
        rowtmp = ctx.enter_context(tc.tile_pool(name="rowtmp", bufs=3))
        ustream = ctx.enter_context(tc.tile_pool(name="ustream", bufs=6))
        utstream = ctx.enter_context(tc.tile_pool(name="utstream", bufs=16))
        utxst = ctx.enter_context(tc.tile_pool(name="utxst", bufs=2))
        zpool = ctx.enter_context(tc.tile_pool(name="zpool", bufs=4))
        xtp = ctx.enter_context(tc.tile_pool(name="xtp", bufs=4))
        dram = ctx.enter_context(tc.tile_pool(name="dram", bufs=1, space="DRAM"))
        ps_p1 = ctx.enter_context(tc.tile_pool(name="ps_p1", bufs=4, space="PSUM"))
        ps_p2 = ctx.enter_context(tc.tile_pool(name="ps_p2", bufs=3, space="PSUM"))
        ps_mm = ctx.enter_context(tc.tile_pool(name="ps_mm", bufs=1, space="PSUM"))

        def p1_tile():
            return ps_p1.tile([128, 512], f32, tag="p1",
                              name=f"p1_{nc.next_id()}")

        def p2_tile():
            return ps_p2.tile([128, 512], f32, tag="p2",
                              name=f"p2_{nc.next_id()}")

        def mm_tile():
            return ps_mm.tile([128, 512], f32, tag="mm",
                              name=f"mm_{nc.next_id()}")

        def wtile(shape, dtype, name):
            return wpool.tile(shape, dtype, tag=name, name=name)

        def rtile(shape, dtype, tag):
            return rowtmp.tile(shape, dtype, tag=tag,
                               name=f"{tag}_{nc.next_id()}")

        def T(out_psum, in_sbuf, identity):
            nc.tensor.matmul(out_psum, in_sbuf, identity, is_transpose=True)

        # ================= constants & weights =================
        ident = wtile([128, 128], f32, "ident")
        make_identity(nc, ident[:])

        ones_row = wtile([1, 128], f32, "ones_row")
        nc.vector.memset(ones_row[:], 1.0)
        eps_col = wtile([128, 1], f32, "eps_col")
        nc.vector.memset(eps_col[:], 1e-5)
        c08_col = wtile([128, 1], f32, "c08_col")
        nc.vector.memset(c08_col[:], 1.0 - LAMBDA_INIT)
        one_col = wtile([128, 1], f32, "one_col")
        nc.vector.memset(one_col[:], 1.0)
        laminit_c = wtile([1, 1], f32, "laminit_c")
        nc.vector.memset(laminit_c[:], LAMBDA_INIT)

        def ldw(name, dram_t, shape, rearr=None, **kw):
            t = wtile(shape, f32, name)
            src = dram_t[:] if rearr is None else dram_t[:].rearrange(rearr, **kw)
            nc.scalar.dma_start(out=t[:], in_=src)
            return t

        few1 = ldw("few1", t_few1, [128, KX, D], "(t p) d -> p t d", p=128)
        few2 = ldw("few2", t_few2, [128, D])
        q1w = ldw("q1w", t_q1w, [128, D])
        k1w = ldw("k1w", t_k1w, [128, D])
        q2w = ldw("q2w", t_q2w, [128, D])
        k2w = ldw("k2w", t_k2w, [128, D])
        vw = ldw("vw", t_vw, [128, D])
        ow = ldw("ow", t_ow, [128, D])
        f1w = ldw("f1w", t_f1w, [128, D])
        f2w = ldw("f2w", t_f2w, [128, D])

        def ldcol(name, dram_t):
            t = wtile([128, 1], f32, name)
            nc.scalar.dma_start(out=t[:],
                                in_=dram_t[:].rearrange("(p x) -> p x", x=1))
            return t

        feb1_c = ldcol("feb1_c", t_feb1)
        feb2_c = ldcol("feb2_c", t_feb2)
        mg_c = ldcol("mg_c", t_mg); mb_c = ldcol("mb_c", t_mb)
        fg_c = ldcol("fg_c", t_fg); fb_c = ldcol("fb_c", t_fb)
        ag_c = ldcol("ag_c", t_ag); ab_c = ldcol("ab_c", t_ab)
        q1b_c = ldcol("q1b_c", t_q1b); q2b_c = ldcol("q2b_c", t_q2b)

        def ldrow(name, dram_t, w=128):
            t = wtile([1, w], f32, name)
            nc.scalar.dma_start(out=t[:],
                                in_=dram_t[:].rearrange("(x p) -> x p", x=1))
            return t

        k1b_r = ldrow("k1b_r", t_k1b); k2b_r = ldrow("k2b_r", t_k2b)
        vb_r = ldrow("vb_r", t_vb); ob_r = ldrow("ob_r", t_ob)
        f1b_r = ldrow("f1b_r", t_f1b)
        lq1_r = ldrow("lq1_r", t_lq1); lk1_r = ldrow("lk1_r", t_lk1)
        lq2_r = ldrow("lq2_r", t_lq2); lk2_r = ldrow("lk2_r", t_lk2)
        kana_r = ldrow("kana_r", t_kana, 10)
        kanb_r = ldrow("kanb_r", t_kanb, 10)
        kbias_r = ldrow("kbias_r", t_kanbias, 1)
        alpha_r = wtile([1, 1], f32, "alpha_r")
        nc.scalar.dma_start(out=alpha_r[:], in_=t_alpha[:])

        def ldbcast(name, dram_t):
            t = wtile([128, D], f32, name)
            nc.scalar.dma_start(out=t[:], in_=dram_t[:].partition_broadcast(128))
            return t

        f2b_B = ldbcast("f2b_B", t_f2b)

        # ---------- scalars: lambda ----------
        srow = wtile([1, 8], f32, "srow")
        nc.vector.memset(srow[:], 0.0)
        tmpr = wtile([1, 128], f32, "tmpr")
        lam1 = wtile([1, 1], f32, "lam1")
        lam2 = wtile([1, 1], f32, "lam2")
        nc.vector.tensor_mul(tmpr[:], lq1_r[:], lk1_r[:])
        nc.vector.tensor_reduce(lam1[:], tmpr[:], axis=AX.X, op=ALU.add)
        nc.scalar.activation(lam1[:], lam1[:], AF.Exp)
        nc.vector.tensor_mul(tmpr[:], lq2_r[:], lk2_r[:])
        nc.vector.tensor_reduce(lam2[:], tmpr[:], axis=AX.X, op=ALU.add)
        nc.scalar.activation(lam2[:], lam2[:], AF.Exp)
        nc.vector.tensor_sub(srow[:, 0:1], lam1[:], lam2[:])
        nc.vector.tensor_add(srow[:, 0:1], srow[:, 0:1], laminit_c[:])  # lam_full
        nc.scalar.mul(srow[:, 1:2], srow[:, 0:1], -1.0)            # -lam_full
        nc.vector.tensor_copy(srow[:, 2:3], alpha_r[:])
        nc.vector.tensor_copy(srow[:, 3:4], kbias_r[:])

        ps_b = mm_tile()[:, :28]
        nc.tensor.matmul(ps_b[:, 0:8], ones_row[:], srow[:],
                         start=True, stop=False)
        nc.tensor.matmul(ps_b[:, 8:18], ones_row[:], kana_r[:],
                         start=False, stop=False)
        nc.tensor.matmul(ps_b[:, 18:28], ones_row[:], kanb_r[:],
                         start=False, stop=True)
        sB = wtile([128, 28], f32, "sB")
        nc.vector.tensor_copy(sB[:], ps_b)
        neglam_c = sB[:, 1:2]
        alpha_c = sB[:, 2:3]
        kbias_c = sB[:, 3:4]

        # ---------- new_e from e (FourierKAN), layout [128, NSUB] ----------
        eT = wtile([128, NSUB], f32, "eT")
        nc.vector.memset(eT[:], 0.0)
        eload = wtile([max(TG_FULL, 1), 128], f32, "eload")
        nc.scalar.dma_start(
            out=eload[:TG_FULL],
            in_=t_e[: TG_FULL * 128].rearrange("(t p) -> t p", p=128))
        pse = mm_tile()[:, :TG_FULL]
        T(pse, eload[:TG_FULL], ident[:TG_FULL, :TG_FULL])
        nc.vector.tensor_copy(eT[:, :TG_FULL], pse)
        if TG_REM > 0:
            erem = wtile([1, TG_REM], f32, "erem")
            nc.scalar.dma_start(
                out=erem[:],
                in_=t_e[TG_FULL * 128:].rearrange("(x p) -> x p", x=1))
            psr = mm_tile()[:TG_REM, :1]
            T(psr, erem[:], ident[:1, :1])
            nc.vector.tensor_copy(eT[:TG_REM, TG_FULL:TG_FULL + 1], psr)

        # Chebyshev recurrence for cos/sin(k*e/pi); theta = e/pi in [0, 0.64]
        s1 = wtile([128, NSUB], f32, "s1")
        nc.scalar.activation(s1[:], eT[:], AF.Sin, scale=1.0 / math.pi)
        c1 = wtile([128, NSUB], f32, "c1")
        nc.vector.tensor_mul(c1[:], s1[:], s1[:])
        nc.scalar.activation(c1[:], c1[:], AF.Sqrt, scale=-1.0, bias=1.0)
        twoc = wtile([128, NSUB], f32, "twoc")
        nc.vector.tensor_add(twoc[:], c1[:], c1[:])

        phi = wtile([128, NSUB], f32, "phi")
        ktmp = wtile([128, NSUB], f32, "ktmp")
        nc.vector.tensor_scalar(phi[:], c1[:], scalar1=sB[:, 8:9],
                                scalar2=None, op0=ALU.mult)
        nc.vector.tensor_scalar(ktmp[:], s1[:], scalar1=sB[:, 18:19],
                                scalar2=None, op0=ALU.mult)
        nc.vector.tensor_add(phi[:], phi[:], ktmp[:])
        cp, sp = c1, s1
        cpp, spp = None, None
        for k in range(2, 11):
            ck = rtile([128, NSUB], f32, "ckt")
            sk = rtile([128, NSUB], f32, "skt")
            nc.vector.tensor_mul(ck[:], twoc[:], cp[:])
            nc.vector.tensor_mul(sk[:], twoc[:], sp[:])
            if k == 2:
                nc.vector.tensor_scalar(ck[:], ck[:], scalar1=one_col[:],
                                        scalar2=None, op0=ALU.subtract)
            else:
                nc.vector.tensor_sub(ck[:], ck[:], cpp[:])
                nc.vector.tensor_sub(sk[:], sk[:], spp[:])
            nc.vector.tensor_scalar(ktmp[:], ck[:],
                                    scalar1=sB[:, 7 + k:8 + k],
                                    scalar2=None, op0=ALU.mult)
            nc.vector.tensor_add(phi[:], phi[:], ktmp[:])
            nc.vector.tensor_scalar(ktmp[:], sk[:],
                                    scalar1=sB[:, 17 + k:18 + k],
                                    scalar2=None, op0=ALU.mult)
            nc.vector.tensor_add(phi[:], phi[:], ktmp[:])
            cpp, spp = cp, sp
            cp, sp = ck, sk
        ne = wtile([128, NSUB], f32, "ne")
        nc.vector.tensor_scalar(ne[:], phi[:], scalar1=kbias_c, op0=ALU.add,
                                scalar2=alpha_c, op1=ALU.mult)

        # ---------- folded weights (LN affine into projections) ----------
        def fold_w(name, w_sb, g_col):
            t = wtile([128, D], f32, name)
            nc.vector.tensor_scalar(t[:], w_sb[:], scalar1=g_col[:],
                                    scalar2=None, op0=ALU.mult)
            return t

        Wk1 = fold_w("Wk1", k1w, mg_c); Wk2 = fold_w("Wk2", k2w, mg_c)
        Wv = fold_w("Wv", vw, mg_c)
        Wq1 = fold_w("Wq1", q1w, mg_c); Wq2 = fold_w("Wq2", q2w, mg_c)
        W1p = fold_w("W1p", f1w, fg_c)
        Wo = wtile([128, D], f32, "Wo")
        nc.vector.tensor_scalar(Wo[:], ow[:], scalar1=ag_c[:], op0=ALU.mult,
                                scalar2=c08_col[:], op1=ALU.mult)

        def fold_b(name, w_sb, beta_col, b_row):
            psb = mm_tile()[:1, :D]
            nc.tensor.matmul(psb, beta_col[:], w_sb[:])
            t = wtile([1, D], f32, name)
            nc.vector.tensor_add(t[:], psb, b_row[:])
            return t

        bk1_r = fold_b("bk1_r", k1w, mb_c, k1b_r)
        bk2_r = fold_b("bk2_r", k2w, mb_c, k2b_r)
        bv_r = fold_b("bv_r", vw, mb_c, vb_r)
        b1p_r = fold_b("b1p_r", f1w, fb_c, f1b_r)
        psq = mm_tile()[:, :1]
        nc.tensor.matmul(psq, q1w[:], mb_c[:])
        bq1_c = wtile([128, 1], f32, "bq1_c")
        nc.vector.tensor_add(bq1_c[:], psq, q1b_c[:])
        psq2 = mm_tile()[:, :1]
        nc.tensor.matmul(psq2, q2w[:], mb_c[:])
        bq2_c = wtile([128, 1], f32, "bq2_c")
        nc.vector.tensor_add(bq2_c[:], psq2, q2b_c[:])
        pso = mm_tile()[:1, :D]
        nc.tensor.matmul(pso, ab_c[:], ow[:])
        bo_r = wtile([1, D], f32, "bo_r")
        nc.vector.tensor_scalar(bo_r[:], pso, scalar1=c08_col[:1],
                                scalar2=None, op0=ALU.mult)
        nc.vector.tensor_add(bo_r[:], bo_r[:], ob_r[:])

        def bcast_row(name, row_sb):
            psb = mm_tile()[:, :D]
            nc.tensor.matmul(psb, ones_row[:], row_sb[:])
            t = wtile([128, D], f32, name)
            nc.vector.tensor_copy(t[:], psb)
            return t

        bk1_B = bcast_row("bk1_B", bk1_r)
        bk2_B = bcast_row("bk2_B", bk2_r)
        bv_B = bcast_row("bv_B", bv_r)
        b1p_B = bcast_row("b1p_B", b1p_r)
        bo_B = bcast_row("bo_B", bo_r)

        # ================= phase A: feature encoder (host-transposed x) ====
        xT_t = []
        for kt in range(KX):
            t = xtp.tile([128, NLOC], f32, tag=f"xt{kt}", name=f"xt{kt}",
                         bufs=1)
            nc.sync.dma_start(out=t[:],
                              in_=t_xT[kt * 128:(kt + 1) * 128, :])
            xT_t.append(t)

        h1T = wtile([128, NLOC], f32, "h1T")
        for io, iw in IBLK:
            psh = p1_tile()[:, :iw]
            for kt in range(KX):
                nc.tensor.matmul(psh, few1[:, kt, :], xT_t[kt][:, io:io + iw],
                                 start=(kt == 0), stop=(kt == KX - 1))
            nc.scalar.activation(h1T[:, io:io + iw], psh, AF.Relu,
                                 bias=feb1_c[:])
        hT = wtile([128, NLOC], f32, "hT")
        for io, iw in IBLK:
            psh = p1_tile()[:, :iw]
            nc.tensor.matmul(psh, few2[:], h1T[:, io:io + iw])
            nc.scalar.add(hT[:, io:io + iw], psh, feb2_c[:])

        h = [wtile([128, D], f32, f"h{r}") for r in range(NR)]
        h16 = [wtile([128, D], bf16, f"h16_{r}") for r in range(NR)]
        hnT = [wtile([128, 128], f32, f"hnT{r}") for r in range(NR)]
        for r, (ro, rw) in enumerate(ROWS):
            pst = p2_tile()[:rw, :D]
            T(pst, hT[:, ro:ro + rw], ident[:])
            nc.vector.tensor_copy(h[r][:rw], pst)
            nc.vector.tensor_copy(h16[r][:rw], pst)

        # ================= phase B: LN + k/v projections + gram =================
        def layer_norm(src_ap, rw, out_ap):
            stats = rtile([128, 6], f32, "stats")
            nc.vector.bn_stats(stats[:rw], src_ap)
            mv = rtile([128, 2], f32, "mv")
            nc.vector.bn_aggr(mv[:rw], stats[:rw])
            rs = rtile([128, 1], f32, "rs")
            nc.scalar.activation(rs[:rw], mv[:rw, 1:2], AF.Sqrt,
                                 bias=eps_col[:rw])
            nc.vector.reciprocal(rs[:rw], rs[:rw])
            nc.vector.tensor_scalar(out_ap, src_ap, scalar1=mv[:rw, 0:1],
                                    op0=ALU.subtract, scalar2=rs[:rw],
                                    op1=ALU.mult)

        gram_ps = [None]

        def emit_phase_b():
            gram_ps[0] = ps_mm.tile([128, 512], f32, tag="mm", name="gram_ps")
            gp = gram_ps[0]
            for r, (ro, rw) in enumerate(ROWS):
                hn = rtile([128, D], f32, "hn")
                layer_norm(h[r][:rw], rw, hn[:rw])
                psT = p2_tile()[:, :rw]
                T(psT, hn[:rw], ident[:rw, :rw])
                nc.vector.tensor_copy(hnT[r][:, :rw], psT)

                k1t = rtile([128, D], f32, "k1t")
                k2t = rtile([128, D], f32, "k2t")
                vt = rtile([128, D], f32, "vt")
                for dst, W, bB in ((k1t, Wk1, bk1_B), (k2t, Wk2, bk2_B),
                                   (vt, Wv, bv_B)):
                    psp = p1_tile()[:rw, :D]
                    nc.tensor.matmul(psp, hnT[r][:, :rw], W[:])
                    nc.vector.tensor_add(dst[:rw], psp, bB[:rw])
                # two disjoint column groups in one bank: safe on HW
                # (per-element has_written), only the sim's zero-region
                # check would object
                nc.tensor.matmul(gp[:, :D], k1t[:rw], vt[:rw],
                                 start=(r == 0), stop=(r == NR - 1),
                                 skip_group_check=True)
                nc.tensor.matmul(gp[:, D:2 * D], k2t[:rw], vt[:rw],
                                 start=(r == 0), stop=(r == NR - 1),
                                 skip_group_check=True)

        def emit_watt():
            # gram rides in AR group 1 (bf16); cast back to f32 on chip
            kv16 = wtile([128, 2 * D], bf16, "kv16")
            nc.scalar.dma_start(out=kv16[:], in_=p1_out[1][:, 4096:4352])
            kv = wtile([128, 2 * D], f32, "kv")
            nc.vector.tensor_copy(kv[:], kv16[:])
            psq1T = mm_tile()[:, :D]
            T(psq1T, Wq1[:], ident[:])
            Wq1T = wtile([128, D], f32, "Wq1T")
            nc.vector.tensor_copy(Wq1T[:], psq1T)
            psq2T = mm_tile()[:, :D]
            T(psq2T, Wq2[:], ident[:])
            Wq2T = wtile([128, D], f32, "Wq2T")
            nc.vector.tensor_copy(Wq2T[:], psq2T)

            ps_w1e = mm_tile()[:, :D]
            nc.tensor.matmul(ps_w1e, Wq1T[:], kv[:, :D])
            Watt = wtile([128, D], f32, "Watt")
            nc.vector.tensor_copy(Watt[:], ps_w1e)
            ps_w2e = mm_tile()[:, :D]
            nc.tensor.matmul(ps_w2e, Wq2T[:], kv[:, D:])
            tmp2 = wtile([128, D], f32, "tmp2")
            nc.vector.tensor_scalar(tmp2[:], ps_w2e, scalar1=neglam_c,
                                    scalar2=None, op0=ALU.mult)
            nc.vector.tensor_add(Watt[:], Watt[:], tmp2[:])

            ps_b1 = mm_tile()[:1, :D]
            nc.tensor.matmul(ps_b1, bq1_c[:], kv[:, :D])
            batt_r = wtile([1, D], f32, "batt_r")
            nc.vector.tensor_copy(batt_r[:], ps_b1)
            ps_b2 = mm_tile()[:1, :D]
            nc.tensor.matmul(ps_b2, bq2_c[:], kv[:, D:])
            tmpb = wtile([1, D], f32, "tmpb")
            nc.vector.tensor_scalar(tmpb[:], ps_b2, scalar1=neglam_c[:1],
                                    scalar2=None, op0=ALU.mult)
            nc.vector.tensor_add(batt_r[:], batt_r[:], tmpb[:])
            batt_B = bcast_row("batt_B", batt_r)
            return Watt, batt_B

        # ================= spectral pipeline =================
        # AR group 1 additionally carries the 2*D gram columns (bf16)
        ar_widths = [4096, 4096 + 2 * D, 1920]
        p1_in, p1_out = [], []
        for g, w in enumerate(ar_widths):
            p1_in.append(dram.tile([128, w], bf16, tag=f"p1in{g}",
                                   name=f"p1in{g}"))
            p1_out.append(dram.tile([128, w], bf16, tag=f"p1out{g}",
                                    name=f"p1out{g}", addr_space=shared_space))
        utxs_t = {}

        def group_of(co):
            for g, (go, gw) in enumerate(AR_GROUPS):
                if go <= co < go + gw:
                    return g, go
            raise AssertionError

        def emit_chunk_pass1(c):
            co, cw = chunks[c]
            g, go = group_of(co)
            if g not in utxs_t:
                utxs_t[g] = utxst.tile([128, 4096 + 2 * D], bf16, tag="utxs",
                                       name=f"utxs{g}")
            cbs = _splits(cw, 512)
            ps1 = [p1_tile()[:, :bw] for bo, bw in cbs]
            for r, (ro, rw) in enumerate(ROWS):
                ut = ustream.tile([128, 2048], bf16, tag="u",
                                  name=f"u{c}_{r}")[:rw, :cw]
                nc.sync.dma_start(out=ut, in_=t_u16[ro:ro + rw, co:co + cw])
                for b, (bo, bw) in enumerate(cbs):
                    nc.tensor.matmul(ps1[b], h16[r][:rw], ut[:, bo:bo + bw],
                                     start=(r == 0), stop=(r == NR - 1))
            lo = co - go
            for b, (bo, bw) in enumerate(cbs):
                nc.vector.tensor_copy(utxs_t[g][:, lo + bo:lo + bo + bw],
                                      ps1[b])

        def emit_ar(g):
            w = ar_widths[g]
            if g == 1:
                # append gram (cast to bf16) to this group's payload
                nc.vector.tensor_copy(utxs_t[1][:, 4096:4096 + 2 * D],
                                      gram_ps[0][:, :2 * D])
            nc.gpsimd.dma_start(out=p1_in[g][:], in_=utxs_t[g][:, :w])
            nc.gpsimd.collective_compute(
                "AllReduce", ALU.add, replica_groups=rg,
                ins=[p1_in[g].opt()], outs=[p1_out[g].opt()])

        ps2_acc = []   # persistent accumulators for henc^T, one per i-block

        def emit_pass2_group(g):
            go, gw = AR_GROUPS[g]
            if not ps2_acc:
                for io, iw in IBLK:
                    ps2_acc.append(p2_tile()[:, :iw])
            for t in range(gw // 128):
                gi = go // 128 + t
                # uT stream tile for this global subtile (no AR dependency)
                utt = utstream.tile([128, NLOC], bf16, tag="ut",
                                    name=f"ut{gi}")
                nc.sync.dma_start(out=utt[:],
                                  in_=t_ut16[gi * 128:(gi + 1) * 128, :])
                # transposed read of the AR'd utx block: zr[k, d], then
                # scale by new_e (per-partition k) in place
                zr = zpool.tile([128, D], bf16, tag="z", name=f"zr{gi}")
                nc.scalar.dma_start(out=zr[:],
                                    in_=p1_out[g][:, t * 128:(t + 1) * 128],
                                    transpose=True)
                nc.vector.tensor_scalar(zr[:], zr[:], scalar1=ne[:, gi:gi + 1],
                                        scalar2=None, op0=ALU.mult)
                first = (gi == 0)
                last = (gi == NSUB - 1)
                for ib, (io, iw) in enumerate(IBLK):
                    nc.tensor.matmul(ps2_acc[ib], zr[:], utt[:, io:io + iw],
                                     start=first, stop=last)

        # ========== pipeline ==========
        emit_chunk_pass1(0)
        emit_phase_b()
        emit_chunk_pass1(1)
        emit_ar(0)
        emit_chunk_pass1(2)
        emit_chunk_pass1(3)
        emit_ar(1)
        emit_pass2_group(0)
        emit_chunk_pass1(4)
        emit_ar(2)
        emit_pass2_group(1)

        # == attention (gram AR completed during early chunks) ==
        Watt, batt_B = emit_watt()
        ha = [wtile([128, D], f32, f"ha{r}") for r in range(NR)]
        s_sbs = []
        for r, (ro, rw) in enumerate(ROWS):
            pss = p1_tile()[:rw, :D]
            nc.tensor.matmul(pss, hnT[r][:, :rw], Watt[:])
            s_sb = wtile([128, D], f32, f"s_sb{r}")
            nc.vector.tensor_add(s_sb[:rw], pss, batt_B[:rw])
            layer_norm(s_sb[:rw], rw, s_sb[:rw])
            s_sbs.append(s_sb)
        for r, (ro, rw) in enumerate(ROWS):
            psT = mm_tile()[:, :rw]
            T(psT, s_sbs[r][:rw], ident[:rw, :rw])
            sT = rtile([128, 128], f32, "sT")
            nc.vector.tensor_copy(sT[:, :rw], psT)
            psa = p1_tile()[:rw, :D]
            nc.tensor.matmul(psa, sT[:, :rw], Wo[:])
            att = rtile([128, D], f32, "att")
            nc.vector.tensor_add(att[:rw], psa, bo_B[:rw])
            nc.vector.tensor_add(ha[r][:rw], h[r][:rw], att[:rw])

        emit_pass2_group(2)

        # ================= residual + FFN =================
        hencT = wtile([128, NLOC], f32, "hencT")
        for ib, (io, iw) in enumerate(IBLK):
            nc.vector.tensor_copy(hencT[:, io:io + iw], ps2_acc[ib])

        mh = [wtile([128, D], f32, f"mh{r}") for r in range(NR)]
        gl = [wtile([128, D], f32, f"gl{r}") for r in range(NR)]
        for r, (ro, rw) in enumerate(ROWS):
            psb = p2_tile()[:rw, :D]
            T(psb, hencT[:, ro:ro + rw], ident[:])
            nc.vector.tensor_add(mh[r][:rw], ha[r][:rw], psb)
            fh = rtile([128, D], f32, "fh")
            layer_norm(mh[r][:rw], rw, fh[:rw])
            psT = p2_tile()[:, :rw]
            T(psT, fh[:rw], ident[:rw, :rw])
            fT = rtile([128, 128], f32, "fT")
            nc.vector.tensor_copy(fT[:, :rw], psT)
            psg = p1_tile()[:rw, :D]
            nc.tensor.matmul(psg, fT[:, :rw], W1p[:])
            nc.vector.tensor_add(gl[r][:rw], psg, b1p_B[:rw])
        for r, (ro, rw) in enumerate(ROWS):
            if sim_gelu:
                # tanh-approx gelu (CoreSim lacks Gelu); HW build uses AF.Gelu
                x3 = rtile([128, D], f32, "x3")
                nc.vector.tensor_mul(x3[:rw], gl[r][:rw], gl[r][:rw])
                nc.vector.tensor_mul(x3[:rw], x3[:rw], gl[r][:rw])
                nc.vector.tensor_scalar(x3[:rw], x3[:rw], scalar1=0.044715,
                                        scalar2=None, op0=ALU.mult)
                nc.vector.tensor_add(x3[:rw], x3[:rw], gl[r][:rw])
                nc.scalar.activation(x3[:rw], x3[:rw], AF.Tanh,
                                     scale=math.sqrt(2.0 / math.pi))
                nc.vector.tensor_scalar(x3[:rw], x3[:rw], scalar1=1.0,
                                        scalar2=0.5, op0=ALU.add, op1=ALU.mult)
                nc.vector.tensor_mul(gl[r][:rw], gl[r][:rw], x3[:rw])
            else:
                nc.scalar.activation(gl[r][:rw], gl[r][:rw], AF.Gelu)
        for r, (ro, rw) in enumerate(ROWS):
            psT2 = p2_tile()[:, :rw]
            T(psT2, gl[r][:rw], ident[:rw, :rw])
            gT = rtile([128, 128], f32, "gT")
            nc.vector.tensor_copy(gT[:, :rw], psT2)
            pso2 = p1_tile()[:rw, :D]
            nc.tensor.matmul(pso2, gT[:, :rw], f2w[:])
            outp = rtile([128, D], f32, "outp")
            nc.vector.tensor_add(outp[:rw], pso2, mh[r][:rw])
            nc.vector.tensor_add(outp[:rw], outp[:rw], f2b_B[:rw])
            nc.gpsimd.dma_start(out=t_out[ro:ro + rw, :], in_=outp[:rw])

    nc.compile()
    return nc


# ==================== host-side entry point ====================

_CACHED = {}


def _get_nc(N=N_FULL, NF=NF_FULL, CORES=CORES_FULL):
    key = (N, NF, CORES)
    if key not in _CACHED:
        _CACHED[key] = build_kernel(N, NF, CORES)
    return _CACHED[key]


def make_in_maps(inputs, N, CORES):
    import ml_dtypes

    bf16 = ml_dtypes.bfloat16
    NLOC = N // CORES
    full = {}
    for k, v in inputs.items():
        if k in ("u", "x"):
            continue
        full[k] = np.ascontiguousarray(np.asarray(v, dtype=np.float32))
    u = np.asarray(inputs["u"], dtype=np.float32)
    x = np.asarray(inputs["x"], dtype=np.float32)
    in_maps = []
    for c in range(CORES):
        rows = slice(c * NLOC, (c + 1) * NLOC)
        u_c = u[rows]
        u16 = np.zeros((NLOC, N_PAD), dtype=bf16)
        u16[:, :N] = u_c.astype(bf16)
        ut16 = np.zeros((N_PAD, NLOC), dtype=bf16)
        ut16[:N, :] = u_c.T.astype(bf16)
        xT = np.ascontiguousarray(x[rows].T)
        m = dict(full)
        m["u16"] = u16
        m["ut16"] = ut16
        m["xT"] = xT
        in_maps.append(m)
    return in_maps


def kernel(**inputs):
    from concourse import bass_utils

    nc = _get_nc()
    in_maps = make_in_maps(inputs, N_FULL, CORES_FULL)
    res = bass_utils.run_bass_kernel_spmd(nc, in_maps,
                                          core_ids=list(range(CORES_FULL)))
    out = np.concatenate([res.results[c]["out"] for c in range(CORES_FULL)],
                         axis=0)
    return out.astype(np.float32)


if __name__ == "__main__":
    build_kernel()
    print("build ok")


# revision 19
# speedup vs baseline: 1.5780x; 1.5780x over previous
"""Trainium2 Bass kernel for nn_NoFoDifformer_FourierKAN (8-core SPMD).

Sharding: u and nodes row-wise across 8 cores (1250 rows each). The [d,d]
K^T V Gram matrices and the chunked u^T h partial sums are all-reduced;
small weights are replicated; per-core row-shard outputs are concatenated
on the host.

The host pre-shards u into TWO bf16 tensors per core: u16 = u[rows,:] for
pass-1 (utx partials) and ut16 = u[rows,:].T for pass-2, both zero-padded
to 79*128 columns/rows. This removes the on-device transpose round-trip
entirely; the device streams each tensor once with large contiguous DMAs.
x is host-pre-transposed so the feature encoder needs no PE transposes of
x. LayerNorm affine params are folded into downstream projection weights.
"""

import math
from contextlib import ExitStack

import numpy as np

N_FULL = 10000
NF_FULL = 512
D = 128
CORES_FULL = 8
N_PAD = 10112                  # 79 * 128
LAMBDA_INIT = 0.2
CHUNK_LIST = [2048, 2048, 2048, 2048, 1920]        # psum chunks, sums to N_PAD
AR_GROUPS = [(0, 4096), (4096, 4096), (8192, 1920)]  # chunks per AllReduce


def _splits(total, step):
    return [(o, min(step, total - o)) for o in range(0, total, step)]


def build_kernel(N=N_FULL, NF=NF_FULL, CORES=CORES_FULL, sim_gelu=False):
    import concourse.bacc as bacc
    import concourse.tile as tile
    from concourse import mybir
    from concourse.masks import make_identity

    dt = mybir.dt
    f32 = dt.float32
    bf16 = dt.bfloat16
    AF = mybir.ActivationFunctionType
    ALU = mybir.AluOpType
    AX = mybir.AxisListType

    NLOC = N // CORES
    assert NLOC * CORES == N
    ROWS = _splits(NLOC, 128)          # per-core row tiles (i)
    NR = len(ROWS)
    KX = NF // 128                     # x feature k-tiles
    assert KX * 128 == NF
    NSUB = N_PAD // 128                # 79 j-subtiles
    chunks, off = [], 0
    for cw in CHUNK_LIST:
        chunks.append((off, cw))
        off += cw
    assert off == N_PAD
    NCH = len(chunks)
    IBLK = _splits(NLOC, 512)          # pass-2 output i blocks
    TG_FULL = N // 128                 # full 128-wide e subtiles
    TG_REM = N - TG_FULL * 128
    rg = [list(range(CORES))]
    shared_space = "Shared" if CORES > 4 else "Local"

    nc = bacc.Bacc("TRN2", target_bir_lowering=False, debug=False,
                   num_devices=CORES)

    # ---------------- DRAM I/O ----------------
    def din(name, shape):
        return nc.dram_tensor(name, list(shape), f32, kind="ExternalInput")

    t_xT = din("xT", (NF, NLOC))
    t_u16 = nc.dram_tensor("u16", [NLOC, N_PAD], bf16, kind="ExternalInput")
    t_ut16 = nc.dram_tensor("ut16", [N_PAD, NLOC], bf16, kind="ExternalInput")
    t_e = din("e", (N,))
    t_few1 = din("fe_w1", (NF, D)); t_feb1 = din("fe_b1", (D,))
    t_few2 = din("fe_w2", (D, D)); t_feb2 = din("fe_b2", (D,))
    t_kana = din("kan_a", (10,)); t_kanb = din("kan_b", (10,))
    t_kanbias = din("kan_bias", (1,)); t_alpha = din("alpha_w", (1, 1))
    t_mg = din("mha_ln_g", (D,)); t_mb = din("mha_ln_b", (D,))
    t_fg = din("ffn_ln_g", (D,)); t_fb = din("ffn_ln_b", (D,))
    t_q1w = din("q1_w", (D, D)); t_q1b = din("q1_b", (D,))
    t_k1w = din("k1_w", (D, D)); t_k1b = din("k1_b", (D,))
    t_q2w = din("q2_w", (D, D)); t_q2b = din("q2_b", (D,))
    t_k2w = din("k2_w", (D, D)); t_k2b = din("k2_b", (D,))
    t_vw = din("v_w", (D, D)); t_vb = din("v_b", (D,))
    t_ag = din("attn_ln_g", (D,)); t_ab = din("attn_ln_b", (D,))
    t_ow = din("out_w", (D, D)); t_ob = din("out_b", (D,))
    t_lq1 = din("lq1", (D,)); t_lk1 = din("lk1", (D,))
    t_lq2 = din("lq2", (D,)); t_lk2 = din("lk2", (D,))
    t_f1w = din("ffn1_w", (D, D)); t_f1b = din("ffn1_b", (D,))
    t_f2w = din("ffn2_w", (D, D)); t_f2b = din("ffn2_b", (D,))
    t_out = nc.dram_tensor("out", [NLOC, D], f32, kind="ExternalOutput")

    with tile.TileContext(nc) as tc, ExitStack() as ctx:
        wpool = ctx.enter_context(tc.tile_pool(name="wpool", bufs=1))
        rowtmp = ctx.enter_context(tc.tile_pool(name="rowtmp", bufs=3))
        ustream = ctx.enter_context(tc.tile_pool(name="ustream", bufs=5))
        utstream = ctx.enter_context(tc.tile_pool(name="utstream", bufs=13))
        utxst = ctx.enter_context(tc.tile_pool(name="utxst", bufs=2))
        utxrd = ctx.enter_context(tc.tile_pool(name="utxrd", bufs=2))
        zpool = ctx.enter_context(tc.tile_pool(name="zpool", bufs=4))
        xtp = ctx.enter_context(tc.tile_pool(name="xtp", bufs=4))
        dram = ctx.enter_context(tc.tile_pool(name="dram", bufs=1, space="DRAM"))
        ps_p1 = ctx.enter_context(tc.tile_pool(name="ps_p1", bufs=4, space="PSUM"))
        ps_p2 = ctx.enter_context(tc.tile_pool(name="ps_p2", bufs=3, space="PSUM"))
        ps_mm = ctx.enter_context(tc.tile_pool(name="ps_mm", bufs=1, space="PSUM"))

        def p1_tile():
            return ps_p1.tile([128, 512], f32, tag="p1",
                              name=f"p1_{nc.next_id()}")

        def p2_tile():
            return ps_p2.tile([128, 512], f32, tag="p2",
                              name=f"p2_{nc.next_id()}")

        def mm_tile():
            return ps_mm.tile([128, 512], f32, tag="mm",
                              name=f"mm_{nc.next_id()}")

        def wtile(shape, dtype, name):
            return wpool.tile(shape, dtype, tag=name, name=name)

        def rtile(shape, dtype, tag):
            return rowtmp.tile(shape, dtype, tag=tag,
                               name=f"{tag}_{nc.next_id()}")

        def T(out_psum, in_sbuf, identity):
            nc.tensor.matmul(out_psum, in_sbuf, identity, is_transpose=True)

        # ================= constants & weights =================
        ident = wtile([128, 128], f32, "ident")
        make_identity(nc, ident[:])
        identb = wtile([128, 128], bf16, "identb")
        make_identity(nc, identb[:])

        # tiny throwaway AllReduce: absorbs the first-collective warmup
        # penalty while the prologue runs
        warm_in = dram.tile([128, 16], bf16, tag="warm_in", name="warm_in")
        warm_out = dram.tile([128, 16], bf16, tag="warm_out", name="warm_out",
                             addr_space=shared_space)
        warm_sb = wtile([128, 16], bf16, "warm_sb")
        nc.vector.memset(warm_sb[:], 0.0)
        nc.gpsimd.dma_start(out=warm_in[:], in_=warm_sb[:])
        nc.gpsimd.collective_compute("AllReduce", ALU.add, replica_groups=rg,
                                     ins=[warm_in.opt()], outs=[warm_out.opt()])

        ones_row = wtile([1, 128], f32, "ones_row")
        nc.vector.memset(ones_row[:], 1.0)
        eps_col = wtile([128, 1], f32, "eps_col")
        nc.vector.memset(eps_col[:], 1e-5)
        c08_col = wtile([128, 1], f32, "c08_col")
        nc.vector.memset(c08_col[:], 1.0 - LAMBDA_INIT)
        one_col = wtile([128, 1], f32, "one_col")
        nc.vector.memset(one_col[:], 1.0)
        laminit_c = wtile([1, 1], f32, "laminit_c")
        nc.vector.memset(laminit_c[:], LAMBDA_INIT)

        def ldw(name, dram_t, shape, rearr=None, **kw):
            t = wtile(shape, f32, name)
            src = dram_t[:] if rearr is None else dram_t[:].rearrange(rearr, **kw)
            nc.scalar.dma_start(out=t[:], in_=src)
            return t

        few1 = ldw("few1", t_few1, [128, KX, D], "(t p) d -> p t d", p=128)
        few2 = ldw("few2", t_few2, [128, D])
        q1w = ldw("q1w", t_q1w, [128, D])
        k1w = ldw("k1w", t_k1w, [128, D])
        q2w = ldw("q2w", t_q2w, [128, D])
        k2w = ldw("k2w", t_k2w, [128, D])
        vw = ldw("vw", t_vw, [128, D])
        ow = ldw("ow", t_ow, [128, D])
        f1w = ldw("f1w", t_f1w, [128, D])
        f2w = ldw("f2w", t_f2w, [128, D])

        def ldcol(name, dram_t):
            t = wtile([128, 1], f32, name)
            nc.scalar.dma_start(out=t[:],
                                in_=dram_t[:].rearrange("(p x) -> p x", x=1))
            return t

        feb1_c = ldcol("feb1_c", t_feb1)
        feb2_c = ldcol("feb2_c", t_feb2)
        mg_c = ldcol("mg_c", t_mg); mb_c = ldcol("mb_c", t_mb)
        fg_c = ldcol("fg_c", t_fg); fb_c = ldcol("fb_c", t_fb)
        ag_c = ldcol("ag_c", t_ag); ab_c = ldcol("ab_c", t_ab)
        q1b_c = ldcol("q1b_c", t_q1b); q2b_c = ldcol("q2b_c", t_q2b)

        def ldrow(name, dram_t, w=128):
            t = wtile([1, w], f32, name)
            nc.scalar.dma_start(out=t[:],
                                in_=dram_t[:].rearrange("(x p) -> x p", x=1))
            return t

        k1b_r = ldrow("k1b_r", t_k1b); k2b_r = ldrow("k2b_r", t_k2b)
        vb_r = ldrow("vb_r", t_vb); ob_r = ldrow("ob_r", t_ob)
        f1b_r = ldrow("f1b_r", t_f1b)
        lq1_r = ldrow("lq1_r", t_lq1); lk1_r = ldrow("lk1_r", t_lk1)
        lq2_r = ldrow("lq2_r", t_lq2); lk2_r = ldrow("lk2_r", t_lk2)
        kana_r = ldrow("kana_r", t_kana, 10)
        kanb_r = ldrow("kanb_r", t_kanb, 10)
        kbias_r = ldrow("kbias_r", t_kanbias, 1)
        alpha_r = wtile([1, 1], f32, "alpha_r")
        nc.scalar.dma_start(out=alpha_r[:], in_=t_alpha[:])

        def ldbcast(name, dram_t):
            t = wtile([128, D], f32, name)
            nc.scalar.dma_start(out=t[:], in_=dram_t[:].partition_broadcast(128))
            return t

        f2b_B = ldbcast("f2b_B", t_f2b)

        # ---------- scalars: lambda ----------
        srow = wtile([1, 8], f32, "srow")
        nc.vector.memset(srow[:], 0.0)
        tmpr = wtile([1, 128], f32, "tmpr")
        lam1 = wtile([1, 1], f32, "lam1")
        lam2 = wtile([1, 1], f32, "lam2")
        nc.vector.tensor_mul(tmpr[:], lq1_r[:], lk1_r[:])
        nc.vector.tensor_reduce(lam1[:], tmpr[:], axis=AX.X, op=ALU.add)
        nc.scalar.activation(lam1[:], lam1[:], AF.Exp)
        nc.vector.tensor_mul(tmpr[:], lq2_r[:], lk2_r[:])
        nc.vector.tensor_reduce(lam2[:], tmpr[:], axis=AX.X, op=ALU.add)
        nc.scalar.activation(lam2[:], lam2[:], AF.Exp)
        nc.vector.tensor_sub(srow[:, 0:1], lam1[:], lam2[:])
        nc.vector.tensor_add(srow[:, 0:1], srow[:, 0:1], laminit_c[:])  # lam_full
        nc.scalar.mul(srow[:, 1:2], srow[:, 0:1], -1.0)            # -lam_full
        nc.vector.tensor_copy(srow[:, 2:3], alpha_r[:])
        nc.vector.tensor_copy(srow[:, 3:4], kbias_r[:])

        ps_b = mm_tile()[:, :28]
        nc.tensor.matmul(ps_b[:, 0:8], ones_row[:], srow[:],
                         start=True, stop=False)
        nc.tensor.matmul(ps_b[:, 8:18], ones_row[:], kana_r[:],
                         start=False, stop=False)
        nc.tensor.matmul(ps_b[:, 18:28], ones_row[:], kanb_r[:],
                         start=False, stop=True)
        sB = wtile([128, 28], f32, "sB")
        nc.vector.tensor_copy(sB[:], ps_b)
        neglam_c = sB[:, 1:2]
        alpha_c = sB[:, 2:3]
        kbias_c = sB[:, 3:4]

        # ---------- new_e from e (FourierKAN), layout [128, NSUB] ----------
        eT = wtile([128, NSUB], f32, "eT")
        nc.vector.memset(eT[:], 0.0)
        eload = wtile([max(TG_FULL, 1), 128], f32, "eload")
        nc.scalar.dma_start(
            out=eload[:TG_FULL],
            in_=t_e[: TG_FULL * 128].rearrange("(t p) -> t p", p=128))
        pse = mm_tile()[:, :TG_FULL]
        T(pse, eload[:TG_FULL], ident[:TG_FULL, :TG_FULL])
        nc.vector.tensor_copy(eT[:, :TG_FULL], pse)
        if TG_REM > 0:
            erem = wtile([1, TG_REM], f32, "erem")
            nc.scalar.dma_start(
                out=erem[:],
                in_=t_e[TG_FULL * 128:].rearrange("(x p) -> x p", x=1))
            psr = mm_tile()[:TG_REM, :1]
            T(psr, erem[:], ident[:1, :1])
            nc.vector.tensor_copy(eT[:TG_REM, TG_FULL:TG_FULL + 1], psr)

        # Chebyshev recurrence for cos/sin(k*e/pi); theta = e/pi in [0, 0.64]
        s1 = wtile([128, NSUB], f32, "s1")
        nc.scalar.activation(s1[:], eT[:], AF.Sin, scale=1.0 / math.pi)
        c1 = wtile([128, NSUB], f32, "c1")
        nc.vector.tensor_mul(c1[:], s1[:], s1[:])
        nc.scalar.activation(c1[:], c1[:], AF.Sqrt, scale=-1.0, bias=1.0)
        twoc = wtile([128, NSUB], f32, "twoc")
        nc.vector.tensor_add(twoc[:], c1[:], c1[:])

        phi = wtile([128, NSUB], f32, "phi")
        ktmp = wtile([128, NSUB], f32, "ktmp")
        nc.vector.tensor_scalar(phi[:], c1[:], scalar1=sB[:, 8:9],
                                scalar2=None, op0=ALU.mult)
        nc.vector.tensor_scalar(ktmp[:], s1[:], scalar1=sB[:, 18:19],
                                scalar2=None, op0=ALU.mult)
        nc.vector.tensor_add(phi[:], phi[:], ktmp[:])
        cp, sp = c1, s1
        cpp, spp = None, None
        for k in range(2, 11):
            ck = rtile([128, NSUB], f32, "ckt")
            sk = rtile([128, NSUB], f32, "skt")
            nc.vector.tensor_mul(ck[:], twoc[:], cp[:])
            nc.vector.tensor_mul(sk[:], twoc[:], sp[:])
            if k == 2:
                nc.vector.tensor_scalar(ck[:], ck[:], scalar1=one_col[:],
                                        scalar2=None, op0=ALU.subtract)
            else:
                nc.vector.tensor_sub(ck[:], ck[:], cpp[:])
                nc.vector.tensor_sub(sk[:], sk[:], spp[:])
            nc.vector.tensor_scalar(ktmp[:], ck[:],
                                    scalar1=sB[:, 7 + k:8 + k],
                                    scalar2=None, op0=ALU.mult)
            nc.vector.tensor_add(phi[:], phi[:], ktmp[:])
            nc.vector.tensor_scalar(ktmp[:], sk[:],
                                    scalar1=sB[:, 17 + k:18 + k],
                                    scalar2=None, op0=ALU.mult)
            nc.vector.tensor_add(phi[:], phi[:], ktmp[:])
            cpp, spp = cp, sp
            cp, sp = ck, sk
        ne = wtile([128, NSUB], f32, "ne")
        nc.vector.tensor_scalar(ne[:], phi[:], scalar1=kbias_c, op0=ALU.add,
                                scalar2=alpha_c, op1=ALU.mult)

        # ---------- folded weights (LN affine into projections) ----------
        def fold_w(name, w_sb, g_col):
            t = wtile([128, D], f32, name)
            nc.vector.tensor_scalar(t[:], w_sb[:], scalar1=g_col[:],
                                    scalar2=None, op0=ALU.mult)
            return t

        Wk1 = fold_w("Wk1", k1w, mg_c); Wk2 = fold_w("Wk2", k2w, mg_c)
        Wv = fold_w("Wv", vw, mg_c)
        Wq1 = fold_w("Wq1", q1w, mg_c); Wq2 = fold_w("Wq2", q2w, mg_c)
        W1p = fold_w("W1p", f1w, fg_c)
        Wo = wtile([128, D], f32, "Wo")
        nc.vector.tensor_scalar(Wo[:], ow[:], scalar1=ag_c[:], op0=ALU.mult,
                                scalar2=c08_col[:], op1=ALU.mult)

        def fold_b(name, w_sb, beta_col, b_row):
            psb = mm_tile()[:1, :D]
            nc.tensor.matmul(psb, beta_col[:], w_sb[:])
            t = wtile([1, D], f32, name)
            nc.vector.tensor_add(t[:], psb, b_row[:])
            return t

        bk1_r = fold_b("bk1_r", k1w, mb_c, k1b_r)
        bk2_r = fold_b("bk2_r", k2w, mb_c, k2b_r)
        bv_r = fold_b("bv_r", vw, mb_c, vb_r)
        b1p_r = fold_b("b1p_r", f1w, fb_c, f1b_r)
        psq = mm_tile()[:, :1]
        nc.tensor.matmul(psq, q1w[:], mb_c[:])
        bq1_c = wtile([128, 1], f32, "bq1_c")
        nc.vector.tensor_add(bq1_c[:], psq, q1b_c[:])
        psq2 = mm_tile()[:, :1]
        nc.tensor.matmul(psq2, q2w[:], mb_c[:])
        bq2_c = wtile([128, 1], f32, "bq2_c")
        nc.vector.tensor_add(bq2_c[:], psq2, q2b_c[:])
        pso = mm_tile()[:1, :D]
        nc.tensor.matmul(pso, ab_c[:], ow[:])
        bo_r = wtile([1, D], f32, "bo_r")
        nc.vector.tensor_scalar(bo_r[:], pso, scalar1=c08_col[:1],
                                scalar2=None, op0=ALU.mult)
        nc.vector.tensor_add(bo_r[:], bo_r[:], ob_r[:])

        def bcast_row(name, row_sb):
            psb = mm_tile()[:, :D]
            nc.tensor.matmul(psb, ones_row[:], row_sb[:])
            t = wtile([128, D], f32, name)
            nc.vector.tensor_copy(t[:], psb)
            return t

        bk1_B = bcast_row("bk1_B", bk1_r)
        bk2_B = bcast_row("bk2_B", bk2_r)
        bv_B = bcast_row("bv_B", bv_r)
        b1p_B = bcast_row("b1p_B", b1p_r)
        bo_B = bcast_row("bo_B", bo_r)

        # ================= phase A: feature encoder (host-transposed x) ====
        xT_t = []
        for kt in range(KX):
            t = xtp.tile([128, NLOC], f32, tag=f"xt{kt}", name=f"xt{kt}",
                         bufs=1)
            nc.sync.dma_start(out=t[:],
                              in_=t_xT[kt * 128:(kt + 1) * 128, :])
            xT_t.append(t)

        h1T = wtile([128, NLOC], f32, "h1T")
        for io, iw in IBLK:
            psh = p1_tile()[:, :iw]
            for kt in range(KX):
                nc.tensor.matmul(psh, few1[:, kt, :], xT_t[kt][:, io:io + iw],
                                 start=(kt == 0), stop=(kt == KX - 1))
            nc.scalar.activation(h1T[:, io:io + iw], psh, AF.Relu,
                                 bias=feb1_c[:])
        hT = wtile([128, NLOC], f32, "hT")
        for io, iw in IBLK:
            psh = p1_tile()[:, :iw]
            nc.tensor.matmul(psh, few2[:], h1T[:, io:io + iw])
            nc.scalar.add(hT[:, io:io + iw], psh, feb2_c[:])

        h = [wtile([128, D], f32, f"h{r}") for r in range(NR)]
        h16 = [wtile([128, D], bf16, f"h16_{r}") for r in range(NR)]
        hnT = [wtile([128, 128], f32, f"hnT{r}") for r in range(NR)]
        for r, (ro, rw) in enumerate(ROWS):
            pst = p2_tile()[:rw, :D]
            T(pst, hT[:, ro:ro + rw], ident[:])
            nc.vector.tensor_copy(h[r][:rw], pst)
            nc.vector.tensor_copy(h16[r][:rw], pst)

        # ================= phase B: LN + k/v projections + gram =================
        def layer_norm(src_ap, rw, out_ap):
            stats = rtile([128, 6], f32, "stats")
            nc.vector.bn_stats(stats[:rw], src_ap)
            mv = rtile([128, 2], f32, "mv")
            nc.vector.bn_aggr(mv[:rw], stats[:rw])
            rs = rtile([128, 1], f32, "rs")
            nc.scalar.activation(rs[:rw], mv[:rw, 1:2], AF.Sqrt,
                                 bias=eps_col[:rw])
            nc.vector.reciprocal(rs[:rw], rs[:rw])
            nc.vector.tensor_scalar(out_ap, src_ap, scalar1=mv[:rw, 0:1],
                                    op0=ALU.subtract, scalar2=rs[:rw],
                                    op1=ALU.mult)

        gram_ps = [None]

        def emit_phase_b():
            gram_ps[0] = ps_mm.tile([128, 512], f32, tag="mm", name="gram_ps")
            gp = gram_ps[0]
            for r, (ro, rw) in enumerate(ROWS):
                hn = rtile([128, D], f32, "hn")
                layer_norm(h[r][:rw], rw, hn[:rw])
                psT = p2_tile()[:, :rw]
                T(psT, hn[:rw], ident[:rw, :rw])
                nc.vector.tensor_copy(hnT[r][:, :rw], psT)

                k1t = rtile([128, D], f32, "k1t")
                k2t = rtile([128, D], f32, "k2t")
                vt = rtile([128, D], f32, "vt")
                for dst, W, bB in ((k1t, Wk1, bk1_B), (k2t, Wk2, bk2_B),
                                   (vt, Wv, bv_B)):
                    psp = p1_tile()[:rw, :D]
                    nc.tensor.matmul(psp, hnT[r][:, :rw], W[:])
                    nc.vector.tensor_add(dst[:rw], psp, bB[:rw])
                # two disjoint column groups in one bank: safe on HW
                # (per-element has_written), only the sim's zero-region
                # check would object
                nc.tensor.matmul(gp[:, :D], k1t[:rw], vt[:rw],
                                 start=(r == 0), stop=(r == NR - 1),
                                 skip_group_check=True)
                nc.tensor.matmul(gp[:, D:2 * D], k2t[:rw], vt[:rw],
                                 start=(r == 0), stop=(r == NR - 1),
                                 skip_group_check=True)

        def emit_watt():
            # gram rides in AR group 1 (bf16); cast back to f32 on chip
            kv16 = wtile([128, 2 * D], bf16, "kv16")
            nc.scalar.dma_start(out=kv16[:], in_=p1_out[1][:, 4096:4352])
            kv = wtile([128, 2 * D], f32, "kv")
            nc.vector.tensor_copy(kv[:], kv16[:])
            psq1T = mm_tile()[:, :D]
            T(psq1T, Wq1[:], ident[:])
            Wq1T = wtile([128, D], f32, "Wq1T")
            nc.vector.tensor_copy(Wq1T[:], psq1T)
            psq2T = mm_tile()[:, :D]
            T(psq2T, Wq2[:], ident[:])
            Wq2T = wtile([128, D], f32, "Wq2T")
            nc.vector.tensor_copy(Wq2T[:], psq2T)

            ps_w1e = mm_tile()[:, :D]
            nc.tensor.matmul(ps_w1e, Wq1T[:], kv[:, :D])
            Watt = wtile([128, D], f32, "Watt")
            nc.vector.tensor_copy(Watt[:], ps_w1e)
            ps_w2e = mm_tile()[:, :D]
            nc.tensor.matmul(ps_w2e, Wq2T[:], kv[:, D:])
            tmp2 = wtile([128, D], f32, "tmp2")
            nc.vector.tensor_scalar(tmp2[:], ps_w2e, scalar1=neglam_c,
                                    scalar2=None, op0=ALU.mult)
            nc.vector.tensor_add(Watt[:], Watt[:], tmp2[:])

            ps_b1 = mm_tile()[:1, :D]
            nc.tensor.matmul(ps_b1, bq1_c[:], kv[:, :D])
            batt_r = wtile([1, D], f32, "batt_r")
            nc.vector.tensor_copy(batt_r[:], ps_b1)
            ps_b2 = mm_tile()[:1, :D]
            nc.tensor.matmul(ps_b2, bq2_c[:], kv[:, D:])
            tmpb = wtile([1, D], f32, "tmpb")
            nc.vector.tensor_scalar(tmpb[:], ps_b2, scalar1=neglam_c[:1],
                                    scalar2=None, op0=ALU.mult)
            nc.vector.tensor_add(batt_r[:], batt_r[:], tmpb[:])
            batt_B = bcast_row("batt_B", batt_r)
            return Watt, batt_B

        # ================= spectral pipeline =================
        # AR group 1 additionally carries the 2*D gram columns (bf16)
        ar_widths = [4096, 4096 + 2 * D, 1920]
        p1_in, p1_out = [], []
        for g, w in enumerate(ar_widths):
            p1_in.append(dram.tile([128, w], bf16, tag=f"p1in{g}",
                                   name=f"p1in{g}"))
            p1_out.append(dram.tile([128, w], bf16, tag=f"p1out{g}",
                                    name=f"p1out{g}", addr_space=shared_space))
        utxs_t = {}
        ut_tiles = {}

        def group_of(co):
            for g, (go, gw) in enumerate(AR_GROUPS):
                if go <= co < go + gw:
                    return g, go
            raise AssertionError

        def emit_chunk_pass1(c):
            co, cw = chunks[c]
            g, go = group_of(co)
            if g not in utxs_t:
                utxs_t[g] = utxst.tile([128, 4096 + 2 * D], bf16, tag="utxs",
                                       name=f"utxs{g}")
            # prefetch pass-2 uT tiles on the scalar HWDGE ring (parallel to
            # the sync ring carrying the u16 stream)
            for gi in range(16 * c, min(16 * (c + 1), NSUB)):
                utt = utstream.tile([128, NLOC], bf16, tag="ut",
                                    name=f"ut{gi}")
                nc.scalar.dma_start(out=utt[:],
                                    in_=t_ut16[gi * 128:(gi + 1) * 128, :])
                ut_tiles[gi] = utt
            cbs = _splits(cw, 512)
            ps1 = [p1_tile()[:, :bw] for bo, bw in cbs]
            for r, (ro, rw) in enumerate(ROWS):
                ut = ustream.tile([128, 2048], bf16, tag="u",
                                  name=f"u{c}_{r}")[:rw, :cw]
                nc.sync.dma_start(out=ut, in_=t_u16[ro:ro + rw, co:co + cw])
                for b, (bo, bw) in enumerate(cbs):
                    nc.tensor.matmul(ps1[b], h16[r][:rw], ut[:, bo:bo + bw],
                                     start=(r == 0), stop=(r == NR - 1))
            lo = co - go
            for b, (bo, bw) in enumerate(cbs):
                nc.vector.tensor_copy(utxs_t[g][:, lo + bo:lo + bo + bw],
                                      ps1[b])

        def emit_ar(g):
            w = ar_widths[g]
            if g == 1:
                # append gram (cast to bf16) to this group's payload
                nc.vector.tensor_copy(utxs_t[1][:, 4096:4096 + 2 * D],
                                      gram_ps[0][:, :2 * D])
            nc.gpsimd.dma_start(out=p1_in[g][:], in_=utxs_t[g][:, :w])
            nc.gpsimd.collective_compute(
                "AllReduce", ALU.add, replica_groups=rg,
                ins=[p1_in[g].opt()], outs=[p1_out[g].opt()])

        ps2_acc = []   # persistent accumulators for henc^T, one per i-block

        def emit_pass2_group(g):
            go, gw = AR_GROUPS[g]
            nsub_g = gw // 128
            if not ps2_acc:
                for io, iw in IBLK:
                    ps2_acc.append(p2_tile()[:, :iw])
            # straight readback of the AR result on the (by now idle) sync ring
            utxr = utxrd.tile([128, 4096], bf16, tag="utxr",
                              name=f"utxr{g}")[:, :gw]
            nc.sync.dma_start(out=utxr, in_=p1_out[g][:, :gw])
            # batches of 4 subtiles: 4 PE transposes into one PSUM tile, then
            # 4 drain copies with the new_e scale fused (per-partition k)
            for b0 in range(0, nsub_g, 4):
                bn = min(4, nsub_g - b0)
                pz = ps_mm.tile([128, 512], bf16, tag="mm",
                                name=f"pz_{nc.next_id()}")
                for b in range(bn):
                    t = b0 + b
                    T(pz[:, b * 128:(b + 1) * 128],
                      utxr[:, t * 128:(t + 1) * 128], identb[:])
                zq = zpool.tile([128, 512], bf16, tag="z",
                                name=f"zq_{nc.next_id()}")
                for b in range(bn):
                    gi = go // 128 + b0 + b
                    nc.vector.tensor_scalar(zq[:, b * 128:(b + 1) * 128],
                                            pz[:, b * 128:(b + 1) * 128],
                                            scalar1=ne[:, gi:gi + 1],
                                            scalar2=None, op0=ALU.mult)
                for b in range(bn):
                    gi = go // 128 + b0 + b
                    first = (gi == 0)
                    last = (gi == NSUB - 1)
                    for ib, (io, iw) in enumerate(IBLK):
                        nc.tensor.matmul(ps2_acc[ib],
                                         zq[:, b * 128:(b + 1) * 128],
                                         ut_tiles[gi][:, io:io + iw],
                                         start=first, stop=last)

        # ========== pipeline ==========
        emit_chunk_pass1(0)
        emit_phase_b()
        emit_chunk_pass1(1)
        emit_ar(0)
        emit_chunk_pass1(2)
        emit_chunk_pass1(3)
        emit_ar(1)
        emit_chunk_pass1(4)
        emit_ar(2)
        emit_pass2_group(0)
        emit_pass2_group(1)

        # == attention (gram AR completed during early chunks) ==
        Watt, batt_B = emit_watt()
        ha = [wtile([128, D], f32, f"ha{r}") for r in range(NR)]
        s_sbs = []
        for r, (ro, rw) in enumerate(ROWS):
            pss = p1_tile()[:rw, :D]
            nc.tensor.matmul(pss, hnT[r][:, :rw], Watt[:])
            s_sb = wtile([128, D], f32, f"s_sb{r}")
            nc.vector.tensor_add(s_sb[:rw], pss, batt_B[:rw])
            layer_norm(s_sb[:rw], rw, s_sb[:rw])
            s_sbs.append(s_sb)
        for r, (ro, rw) in enumerate(ROWS):
            psT = mm_tile()[:, :rw]
            T(psT, s_sbs[r][:rw], ident[:rw, :rw])
            sT = rtile([128, 128], f32, "sT")
            nc.vector.tensor_copy(sT[:, :rw], psT)
            psa = p1_tile()[:rw, :D]
            nc.tensor.matmul(psa, sT[:, :rw], Wo[:])
            att = rtile([128, D], f32, "att")
            nc.vector.tensor_add(att[:rw], psa, bo_B[:rw])
            nc.vector.tensor_add(ha[r][:rw], h[r][:rw], att[:rw])

        emit_pass2_group(2)

        # ================= residual + FFN =================
        hencT = wtile([128, NLOC], f32, "hencT")
        for ib, (io, iw) in enumerate(IBLK):
            nc.vector.tensor_copy(hencT[:, io:io + iw], ps2_acc[ib])

        mh = [wtile([128, D], f32, f"mh{r}") for r in range(NR)]
        gl = [wtile([128, D], f32, f"gl{r}") for r in range(NR)]
        for r, (ro, rw) in enumerate(ROWS):
            psb = p2_tile()[:rw, :D]
            T(psb, hencT[:, ro:ro + rw], ident[:])
            nc.vector.tensor_add(mh[r][:rw], ha[r][:rw], psb)
            fh = rtile([128, D], f32, "fh")
            layer_norm(mh[r][:rw], rw, fh[:rw])
            psT = p2_tile()[:, :rw]
            T(psT, fh[:rw], ident[:rw, :rw])
            fT = rtile([128, 128], f32, "fT")
            nc.vector.tensor_copy(fT[:, :rw], psT)
            psg = p1_tile()[:rw, :D]
            nc.tensor.matmul(psg, fT[:, :rw], W1p[:])
            nc.vector.tensor_add(gl[r][:rw], psg, b1p_B[:rw])
        for r, (ro, rw) in enumerate(ROWS):
            if sim_gelu:
                # tanh-approx gelu (CoreSim lacks Gelu); HW build uses AF.Gelu
                x3 = rtile([128, D], f32, "x3")
                nc.vector.tensor_mul(x3[:rw], gl[r][:rw], gl[r][:rw])
                nc.vector.tensor_mul(x3[:rw], x3[:rw], gl[r][:rw])
                nc.vector.tensor_scalar(x3[:rw], x3[:rw], scalar1=0.044715,
                                        scalar2=None, op0=ALU.mult)
                nc.vector.tensor_add(x3[:rw], x3[:rw], gl[r][:rw])
                nc.scalar.activation(x3[:rw], x3[:rw], AF.Tanh,
                                     scale=math.sqrt(2.0 / math.pi))
                nc.vector.tensor_scalar(x3[:rw], x3[:rw], scalar1=1.0,
                                        scalar2=0.5, op0=ALU.add, op1=ALU.mult)
                nc.vector.tensor_mul(gl[r][:rw], gl[r][:rw], x3[:rw])
            else:
                nc.scalar.activation(gl[r][:rw], gl[r][:rw], AF.Gelu)
        for r, (ro, rw) in enumerate(ROWS):
            psT2 = p2_tile()[:, :rw]
            T(psT2, gl[r][:rw], ident[:rw, :rw])
            gT = rtile([128, 128], f32, "gT")
            nc.vector.tensor_copy(gT[:, :rw], psT2)
            pso2 = p1_tile()[:rw, :D]
            nc.tensor.matmul(pso2, gT[:, :rw], f2w[:])
            outp = rtile([128, D], f32, "outp")
            nc.vector.tensor_add(outp[:rw], pso2, mh[r][:rw])
            nc.vector.tensor_add(outp[:rw], outp[:rw], f2b_B[:rw])
            nc.gpsimd.dma_start(out=t_out[ro:ro + rw, :], in_=outp[:rw])

    nc.compile()
    return nc


# ==================== host-side entry point ====================

_CACHED = {}


def _get_nc(N=N_FULL, NF=NF_FULL, CORES=CORES_FULL):
    key = (N, NF, CORES)
    if key not in _CACHED:
        _CACHED[key] = build_kernel(N, NF, CORES)
    return _CACHED[key]


def make_in_maps(inputs, N, CORES):
    import ml_dtypes

    bf16 = ml_dtypes.bfloat16
    NLOC = N // CORES
    full = {}
    for k, v in inputs.items():
        if k in ("u", "x"):
            continue
        full[k] = np.ascontiguousarray(np.asarray(v, dtype=np.float32))
    u = np.asarray(inputs["u"], dtype=np.float32)
    x = np.asarray(inputs["x"], dtype=np.float32)
    in_maps = []
    for c in range(CORES):
        rows = slice(c * NLOC, (c + 1) * NLOC)
        u_c = u[rows]
        u16 = np.zeros((NLOC, N_PAD), dtype=bf16)
        u16[:, :N] = u_c.astype(bf16)
        ut16 = np.zeros((N_PAD, NLOC), dtype=bf16)
        ut16[:N, :] = u_c.T.astype(bf16)
        xT = np.ascontiguousarray(x[rows].T)
        m = dict(full)
        m["u16"] = u16
        m["ut16"] = ut16
        m["xT"] = xT
        in_maps.append(m)
    return in_maps


def kernel(**inputs):
    from concourse import bass_utils

    nc = _get_nc()
    in_maps = make_in_maps(inputs, N_FULL, CORES_FULL)
    res = bass_utils.run_bass_kernel_spmd(nc, in_maps,
                                          core_ids=list(range(CORES_FULL)))
    out = np.concatenate([res.results[c]["out"] for c in range(CORES_FULL)],
                         axis=0)
    return out.astype(np.float32)


if __name__ == "__main__":
    build_kernel()
    print("build ok")


# revision 33
# speedup vs baseline: 1.6525x; 1.0472x over previous
"""Trainium2 Bass kernel for nn_NoFoDifformer_FourierKAN (8-core SPMD).

Sharding: u and nodes row-wise across 8 cores (1250 rows each). The [d,d]
K^T V Gram matrices and the chunked u^T h partial sums are all-reduced;
small weights are replicated; per-core row-shard outputs are concatenated
on the host.

The host pre-shards u into TWO bf16 tensors per core: u16 = u[rows,:] for
pass-1 (utx partials) and ut16 = u[rows,:].T for pass-2, both zero-padded
to 79*128 columns/rows. This removes the on-device transpose round-trip
entirely; the device streams each tensor once with large contiguous DMAs.
x is host-pre-transposed so the feature encoder needs no PE transposes of
x. LayerNorm affine params are folded into downstream projection weights.
"""

import math
from contextlib import ExitStack

import numpy as np

N_FULL = 10000
NF_FULL = 512
D = 128
CORES_FULL = 8
N_PAD = 10112                  # 79 * 128
LAMBDA_INIT = 0.2
CHUNK_LIST = [2048, 2048, 2048, 2048, 1920]        # psum chunks, sums to N_PAD
# AllReduce groups (offset, width); the last one also carries the gram
AR_GROUPS = [(0, 2048), (2048, 2048), (4096, 4096), (8192, 1920)]


def _splits(total, step):
    return [(o, min(step, total - o)) for o in range(0, total, step)]


def build_kernel(N=N_FULL, NF=NF_FULL, CORES=CORES_FULL, sim_gelu=False):
    import concourse.bacc as bacc
    import concourse.tile as tile
    from concourse import mybir
    from concourse.masks import make_identity

    dt = mybir.dt
    f32 = dt.float32
    bf16 = dt.bfloat16
    AF = mybir.ActivationFunctionType
    ALU = mybir.AluOpType
    AX = mybir.AxisListType

    NLOC = N // CORES
    assert NLOC * CORES == N
    ROWS = _splits(NLOC, 128)          # per-core row tiles (i)
    NR = len(ROWS)
    KX = NF // 128                     # x feature k-tiles
    assert KX * 128 == NF
    NSUB = N_PAD // 128                # 79 j-subtiles
    chunks, off = [], 0
    for cw in CHUNK_LIST:
        chunks.append((off, cw))
        off += cw
    assert off == N_PAD
    NCH = len(chunks)
    IBLK = _splits(NLOC, 512)          # pass-2 output i blocks
    TG_FULL = N // 128                 # full 128-wide e subtiles
    TG_REM = N - TG_FULL * 128
    rg = [list(range(CORES))]
    shared_space = "Shared" if CORES > 4 else "Local"

    nc = bacc.Bacc("TRN2", target_bir_lowering=False, debug=False,
                   num_devices=CORES)

    # ---------------- DRAM I/O ----------------
    def din(name, shape):
        return nc.dram_tensor(name, list(shape), f32, kind="ExternalInput")

    t_xT = nc.dram_tensor("xT", [NF, NLOC], bf16, kind="ExternalInput")
    t_u16 = nc.dram_tensor("u16", [NLOC, N_PAD], bf16, kind="ExternalInput")
    t_ut16 = nc.dram_tensor("ut16", [N_PAD, NLOC], bf16, kind="ExternalInput")
    t_e = din("e", (N,))
    t_few1 = din("fe_w1", (NF, D)); t_feb1 = din("fe_b1", (D,))
    t_few2 = din("fe_w2", (D, D)); t_feb2 = din("fe_b2", (D,))
    t_kana = din("kan_a", (10,)); t_kanb = din("kan_b", (10,))
    t_kanbias = din("kan_bias", (1,)); t_alpha = din("alpha_w", (1, 1))
    t_mg = din("mha_ln_g", (D,)); t_mb = din("mha_ln_b", (D,))
    t_fg = din("ffn_ln_g", (D,)); t_fb = din("ffn_ln_b", (D,))
    t_q1w = din("q1_w", (D, D)); t_q1b = din("q1_b", (D,))
    t_k1w = din("k1_w", (D, D)); t_k1b = din("k1_b", (D,))
    t_q2w = din("q2_w", (D, D)); t_q2b = din("q2_b", (D,))
    t_k2w = din("k2_w", (D, D)); t_k2b = din("k2_b", (D,))
    t_vw = din("v_w", (D, D)); t_vb = din("v_b", (D,))
    t_ag = din("attn_ln_g", (D,)); t_ab = din("attn_ln_b", (D,))
    t_ow = din("out_w", (D, D)); t_ob = din("out_b", (D,))
    t_lq1 = din("lq1", (D,)); t_lk1 = din("lk1", (D,))
    t_lq2 = din("lq2", (D,)); t_lk2 = din("lk2", (D,))
    t_f1w = din("ffn1_w", (D, D)); t_f1b = din("ffn1_b", (D,))
    t_f2w = din("ffn2_w", (D, D)); t_f2b = din("ffn2_b", (D,))
    t_out = nc.dram_tensor("out", [NLOC, D], f32, kind="ExternalOutput")

    with tile.TileContext(nc) as tc, ExitStack() as ctx:
        wpool = ctx.enter_context(tc.tile_pool(name="wpool", bufs=1))
        rowtmp = ctx.enter_context(tc.tile_pool(name="rowtmp", bufs=3))
        ustream = ctx.enter_context(tc.tile_pool(name="ustream", bufs=5))
        utstream = ctx.enter_context(tc.tile_pool(name="utstream", bufs=13))
        utxst = ctx.enter_context(tc.tile_pool(name="utxst", bufs=2))
        utxrd = ctx.enter_context(tc.tile_pool(name="utxrd", bufs=2))
        zpool = ctx.enter_context(tc.tile_pool(name="zpool", bufs=4))
        xtp = ctx.enter_context(tc.tile_pool(name="xtp", bufs=4))
        dram = ctx.enter_context(tc.tile_pool(name="dram", bufs=1, space="DRAM"))
        ps_p1 = ctx.enter_context(tc.tile_pool(name="ps_p1", bufs=4, space="PSUM"))
        ps_p2 = ctx.enter_context(tc.tile_pool(name="ps_p2", bufs=3, space="PSUM"))
        ps_mm = ctx.enter_context(tc.tile_pool(name="ps_mm", bufs=1, space="PSUM"))

        def p1_tile():
            return ps_p1.tile([128, 512], f32, tag="p1",
                              name=f"p1_{nc.next_id()}")

        def p2_tile():
            return ps_p2.tile([128, 512], f32, tag="p2",
                              name=f"p2_{nc.next_id()}")

        def mm_tile():
            return ps_mm.tile([128, 512], f32, tag="mm",
                              name=f"mm_{nc.next_id()}")

        def wtile(shape, dtype, name):
            return wpool.tile(shape, dtype, tag=name, name=name)

        def rtile(shape, dtype, tag):
            return rowtmp.tile(shape, dtype, tag=tag,
                               name=f"{tag}_{nc.next_id()}")

        def T(out_psum, in_sbuf, identity):
            nc.tensor.matmul(out_psum, in_sbuf, identity, is_transpose=True)

        # ================= constants & weights =================
        ident = wtile([128, 128], f32, "ident")
        make_identity(nc, ident[:])
        identb = wtile([128, 128], bf16, "identb")
        make_identity(nc, identb[:])

        # tiny throwaway AllReduce: absorbs the first-collective warmup
        # penalty while the prologue runs
        warm_in = dram.tile([128, 16], bf16, tag="warm_in", name="warm_in")
        warm_out = dram.tile([128, 16], bf16, tag="warm_out", name="warm_out",
                             addr_space=shared_space)
        warm_sb = wtile([128, 16], bf16, "warm_sb")
        nc.vector.memset(warm_sb[:], 0.0)
        nc.gpsimd.dma_start(out=warm_in[:], in_=warm_sb[:])
        nc.gpsimd.collective_compute("AllReduce", ALU.add, replica_groups=rg,
                                     ins=[warm_in.opt()], outs=[warm_out.opt()])

        ones_row = wtile([1, 128], f32, "ones_row")
        nc.vector.memset(ones_row[:], 1.0)
        eps_col = wtile([128, 1], f32, "eps_col")
        nc.vector.memset(eps_col[:], 1e-5)
        c08_col = wtile([128, 1], f32, "c08_col")
        nc.vector.memset(c08_col[:], 1.0 - LAMBDA_INIT)
        one_col = wtile([128, 1], f32, "one_col")
        nc.vector.memset(one_col[:], 1.0)
        laminit_c = wtile([1, 1], f32, "laminit_c")
        nc.vector.memset(laminit_c[:], LAMBDA_INIT)

        def ldw(name, dram_t, shape, rearr=None, **kw):
            t = wtile(shape, f32, name)
            src = dram_t[:] if rearr is None else dram_t[:].rearrange(rearr, **kw)
            nc.scalar.dma_start(out=t[:], in_=src)
            return t

        few1 = ldw("few1", t_few1, [128, KX, D], "(t p) d -> p t d", p=128)
        few1b = wtile([128, KX, D], bf16, "few1b")
        nc.vector.tensor_copy(few1b[:], few1[:])
        few2 = ldw("few2", t_few2, [128, D])
        q1w = ldw("q1w", t_q1w, [128, D])
        k1w = ldw("k1w", t_k1w, [128, D])
        q2w = ldw("q2w", t_q2w, [128, D])
        k2w = ldw("k2w", t_k2w, [128, D])
        vw = ldw("vw", t_vw, [128, D])
        ow = ldw("ow", t_ow, [128, D])
        f1w = ldw("f1w", t_f1w, [128, D])
        f2w = ldw("f2w", t_f2w, [128, D])

        def ldcol(name, dram_t):
            t = wtile([128, 1], f32, name)
            nc.scalar.dma_start(out=t[:],
                                in_=dram_t[:].rearrange("(p x) -> p x", x=1))
            return t

        feb1_c = ldcol("feb1_c", t_feb1)
        feb2_c = ldcol("feb2_c", t_feb2)
        mg_c = ldcol("mg_c", t_mg); mb_c = ldcol("mb_c", t_mb)
        fg_c = ldcol("fg_c", t_fg); fb_c = ldcol("fb_c", t_fb)
        ag_c = ldcol("ag_c", t_ag); ab_c = ldcol("ab_c", t_ab)
        q1b_c = ldcol("q1b_c", t_q1b); q2b_c = ldcol("q2b_c", t_q2b)

        def ldrow(name, dram_t, w=128):
            t = wtile([1, w], f32, name)
            nc.scalar.dma_start(out=t[:],
                                in_=dram_t[:].rearrange("(x p) -> x p", x=1))
            return t

        k1b_r = ldrow("k1b_r", t_k1b); k2b_r = ldrow("k2b_r", t_k2b)
        vb_r = ldrow("vb_r", t_vb); ob_r = ldrow("ob_r", t_ob)
        f1b_r = ldrow("f1b_r", t_f1b)
        lq1_r = ldrow("lq1_r", t_lq1); lk1_r = ldrow("lk1_r", t_lk1)
        lq2_r = ldrow("lq2_r", t_lq2); lk2_r = ldrow("lk2_r", t_lk2)
        kana_r = ldrow("kana_r", t_kana, 10)
        kanb_r = ldrow("kanb_r", t_kanb, 10)
        kbias_r = ldrow("kbias_r", t_kanbias, 1)
        alpha_r = wtile([1, 1], f32, "alpha_r")
        nc.scalar.dma_start(out=alpha_r[:], in_=t_alpha[:])

        def ldbcast(name, dram_t):
            t = wtile([128, D], f32, name)
            nc.scalar.dma_start(out=t[:], in_=dram_t[:].partition_broadcast(128))
            return t

        f2b_B = ldbcast("f2b_B", t_f2b)

        # ---------- scalars: lambda ----------
        srow = wtile([1, 8], f32, "srow")
        nc.vector.memset(srow[:], 0.0)
        tmpr = wtile([1, 128], f32, "tmpr")
        lam1 = wtile([1, 1], f32, "lam1")
        lam2 = wtile([1, 1], f32, "lam2")
        nc.vector.tensor_mul(tmpr[:], lq1_r[:], lk1_r[:])
        nc.vector.tensor_reduce(lam1[:], tmpr[:], axis=AX.X, op=ALU.add)
        nc.scalar.activation(lam1[:], lam1[:], AF.Exp)
        nc.vector.tensor_mul(tmpr[:], lq2_r[:], lk2_r[:])
        nc.vector.tensor_reduce(lam2[:], tmpr[:], axis=AX.X, op=ALU.add)
        nc.scalar.activation(lam2[:], lam2[:], AF.Exp)
        nc.vector.tensor_sub(srow[:, 0:1], lam1[:], lam2[:])
        nc.vector.tensor_add(srow[:, 0:1], srow[:, 0:1], laminit_c[:])  # lam_full
        nc.scalar.mul(srow[:, 1:2], srow[:, 0:1], -1.0)            # -lam_full
        nc.vector.tensor_copy(srow[:, 2:3], alpha_r[:])
        nc.vector.tensor_copy(srow[:, 3:4], kbias_r[:])

        ps_b = mm_tile()[:, :28]
        nc.tensor.matmul(ps_b[:, 0:8], ones_row[:], srow[:],
                         start=True, stop=False)
        nc.tensor.matmul(ps_b[:, 8:18], ones_row[:], kana_r[:],
                         start=False, stop=False)
        nc.tensor.matmul(ps_b[:, 18:28], ones_row[:], kanb_r[:],
                         start=False, stop=True)
        sB = wtile([128, 28], f32, "sB")
        nc.vector.tensor_copy(sB[:], ps_b)
        neglam_c = sB[:, 1:2]
        alpha_c = sB[:, 2:3]
        kbias_c = sB[:, 3:4]

        # ---------- new_e from e (FourierKAN), layout [128, NSUB] ----------
        eT = wtile([128, NSUB], f32, "eT")
        nc.vector.memset(eT[:], 0.0)
        eload = wtile([max(TG_FULL, 1), 128], f32, "eload")
        nc.scalar.dma_start(
            out=eload[:TG_FULL],
            in_=t_e[: TG_FULL * 128].rearrange("(t p) -> t p", p=128))
        pse = mm_tile()[:, :TG_FULL]
        T(pse, eload[:TG_FULL], ident[:TG_FULL, :TG_FULL])
        nc.vector.tensor_copy(eT[:, :TG_FULL], pse)
        if TG_REM > 0:
            erem = wtile([1, TG_REM], f32, "erem")
            nc.scalar.dma_start(
                out=erem[:],
                in_=t_e[TG_FULL * 128:].rearrange("(x p) -> x p", x=1))
            psr = mm_tile()[:TG_REM, :1]
            T(psr, erem[:], ident[:1, :1])
            nc.vector.tensor_copy(eT[:TG_REM, TG_FULL:TG_FULL + 1], psr)

        # Chebyshev recurrence for cos/sin(k*e/pi); theta = e/pi in [0, 0.64]
        s1 = wtile([128, NSUB], f32, "s1")
        nc.scalar.activation(s1[:], eT[:], AF.Sin, scale=1.0 / math.pi)
        c1 = wtile([128, NSUB], f32, "c1")
        nc.vector.tensor_mul(c1[:], s1[:], s1[:])
        nc.scalar.activation(c1[:], c1[:], AF.Sqrt, scale=-1.0, bias=1.0)
        twoc = wtile([128, NSUB], f32, "twoc")
        nc.vector.tensor_add(twoc[:], c1[:], c1[:])

        phi = wtile([128, NSUB], f32, "phi")
        ktmp = wtile([128, NSUB], f32, "ktmp")
        nc.vector.tensor_scalar(phi[:], c1[:], scalar1=sB[:, 8:9],
                                scalar2=None, op0=ALU.mult)
        nc.vector.tensor_scalar(ktmp[:], s1[:], scalar1=sB[:, 18:19],
                                scalar2=None, op0=ALU.mult)
        nc.vector.tensor_add(phi[:], phi[:], ktmp[:])
        cp, sp = c1, s1
        cpp, spp = None, None
        for k in range(2, 11):
            ck = rtile([128, NSUB], f32, "ckt")
            sk = rtile([128, NSUB], f32, "skt")
            nc.vector.tensor_mul(ck[:], twoc[:], cp[:])
            nc.vector.tensor_mul(sk[:], twoc[:], sp[:])
            if k == 2:
                nc.vector.tensor_scalar(ck[:], ck[:], scalar1=one_col[:],
                                        scalar2=None, op0=ALU.subtract)
            else:
                nc.vector.tensor_sub(ck[:], ck[:], cpp[:])
                nc.vector.tensor_sub(sk[:], sk[:], spp[:])
            nc.vector.tensor_scalar(ktmp[:], ck[:],
                                    scalar1=sB[:, 7 + k:8 + k],
                                    scalar2=None, op0=ALU.mult)
            nc.vector.tensor_add(phi[:], phi[:], ktmp[:])
            nc.vector.tensor_scalar(ktmp[:], sk[:],
                                    scalar1=sB[:, 17 + k:18 + k],
                                    scalar2=None, op0=ALU.mult)
            nc.vector.tensor_add(phi[:], phi[:], ktmp[:])
            cpp, spp = cp, sp
            cp, sp = ck, sk
        ne = wtile([128, NSUB], f32, "ne")
        nc.vector.tensor_scalar(ne[:], phi[:], scalar1=kbias_c, op0=ALU.add,
                                scalar2=alpha_c, op1=ALU.mult)

        # ---------- folded weights (LN affine into projections) ----------
        def fold_w(name, w_sb, g_col):
            t = wtile([128, D], f32, name)
            nc.vector.tensor_scalar(t[:], w_sb[:], scalar1=g_col[:],
                                    scalar2=None, op0=ALU.mult)
            return t

        Wk1 = fold_w("Wk1", k1w, mg_c); Wk2 = fold_w("Wk2", k2w, mg_c)
        Wv = fold_w("Wv", vw, mg_c)
        Wq1 = fold_w("Wq1", q1w, mg_c); Wq2 = fold_w("Wq2", q2w, mg_c)
        W1p = fold_w("W1p", f1w, fg_c)
        Wo = wtile([128, D], f32, "Wo")
        nc.vector.tensor_scalar(Wo[:], ow[:], scalar1=ag_c[:], op0=ALU.mult,
                                scalar2=c08_col[:], op1=ALU.mult)

        def fold_b(name, w_sb, beta_col, b_row):
            psb = mm_tile()[:1, :D]
            nc.tensor.matmul(psb, beta_col[:], w_sb[:])
            t = wtile([1, D], f32, name)
            nc.vector.tensor_add(t[:], psb, b_row[:])
            return t

        bk1_r = fold_b("bk1_r", k1w, mb_c, k1b_r)
        bk2_r = fold_b("bk2_r", k2w, mb_c, k2b_r)
        bv_r = fold_b("bv_r", vw, mb_c, vb_r)
        b1p_r = fold_b("b1p_r", f1w, fb_c, f1b_r)
        psq = mm_tile()[:, :1]
        nc.tensor.matmul(psq, q1w[:], mb_c[:])
        bq1_c = wtile([128, 1], f32, "bq1_c")
        nc.vector.tensor_add(bq1_c[:], psq, q1b_c[:])
        psq2 = mm_tile()[:, :1]
        nc.tensor.matmul(psq2, q2w[:], mb_c[:])
        bq2_c = wtile([128, 1], f32, "bq2_c")
        nc.vector.tensor_add(bq2_c[:], psq2, q2b_c[:])
        pso = mm_tile()[:1, :D]
        nc.tensor.matmul(pso, ab_c[:], ow[:])
        bo_r = wtile([1, D], f32, "bo_r")
        nc.vector.tensor_scalar(bo_r[:], pso, scalar1=c08_col[:1],
                                scalar2=None, op0=ALU.mult)
        nc.vector.tensor_add(bo_r[:], bo_r[:], ob_r[:])

        def bcast_row(name, row_sb):
            psb = mm_tile()[:, :D]
            nc.tensor.matmul(psb, ones_row[:], row_sb[:])
            t = wtile([128, D], f32, name)
            nc.vector.tensor_copy(t[:], psb)
            return t

        bk1_B = bcast_row("bk1_B", bk1_r)
        bk2_B = bcast_row("bk2_B", bk2_r)
        bv_B = bcast_row("bv_B", bv_r)
        b1p_B = bcast_row("b1p_B", b1p_r)
        bo_B = bcast_row("bo_B", bo_r)

        # ================= phase A: feature encoder (host-transposed x) ====
        xT_t = []
        for kt in range(KX):
            t = xtp.tile([128, NLOC], bf16, tag=f"xt{kt}", name=f"xt{kt}",
                         bufs=1)
            nc.sync.dma_start(out=t[:],
                              in_=t_xT[kt * 128:(kt + 1) * 128, :])
            xT_t.append(t)

        h1T = wtile([128, NLOC], f32, "h1T")
        for io, iw in IBLK:
            psh = p1_tile()[:, :iw]
            for kt in range(KX):
                nc.tensor.matmul(psh, few1b[:, kt, :], xT_t[kt][:, io:io + iw],
                                 start=(kt == 0), stop=(kt == KX - 1))
            nc.scalar.activation(h1T[:, io:io + iw], psh, AF.Relu,
                                 bias=feb1_c[:])
        hT = wtile([128, NLOC], f32, "hT")
        for io, iw in IBLK:
            psh = p1_tile()[:, :iw]
            nc.tensor.matmul(psh, few2[:], h1T[:, io:io + iw])
            nc.vector.tensor_scalar(hT[:, io:io + iw], psh,
                                    scalar1=feb2_c[:], scalar2=None,
                                    op0=ALU.add)

        h = [wtile([128, D], f32, f"h{r}") for r in range(NR)]
        h16 = [wtile([128, D], bf16, f"h16_{r}") for r in range(NR)]
        hnT = [wtile([128, 128], f32, f"hnT{r}") for r in range(NR)]
        for r, (ro, rw) in enumerate(ROWS):
            pst = p2_tile()[:rw, :D]
            T(pst, hT[:, ro:ro + rw], ident[:])
            nc.vector.tensor_copy(h[r][:rw], pst)
            nc.vector.tensor_copy(h16[r][:rw], pst)

        # ================= phase B: LN + k/v projections + gram =================
        def layer_norm_batch(items, tagbase):
            # stage-major LN over several row tiles: engines pipeline instead
            # of ping-ponging per tile
            n = len(items)
            mv = wtile([128, 2 * n], f32, f"mv_{tagbase}")
            rs = wtile([128, n], f32, f"rs_{tagbase}")
            for j, (src, rw, out) in enumerate(items):
                stats = rtile([128, 6], f32, "stats")
                nc.vector.bn_stats(stats[:rw], src)
                nc.vector.bn_aggr(mv[:rw, 2 * j:2 * j + 2], stats[:rw])
            for j, (src, rw, out) in enumerate(items):
                nc.scalar.activation(rs[:rw, j:j + 1], mv[:rw, 2 * j + 1:2 * j + 2],
                                     AF.Sqrt, bias=eps_col[:rw])
                nc.vector.reciprocal(rs[:rw, j:j + 1], rs[:rw, j:j + 1])
            for j, (src, rw, out) in enumerate(items):
                nc.vector.tensor_scalar(out, src, scalar1=mv[:rw, 2 * j:2 * j + 1],
                                        op0=ALU.subtract,
                                        scalar2=rs[:rw, j:j + 1], op1=ALU.mult)

        gram_ps = [None]

        def emit_phase_b():
            gram_ps[0] = ps_mm.tile([128, 512], f32, tag="mm", name="gram_ps")
            gp = gram_ps[0]
            hn = [wtile([128, D], f32, f"hn{r}") for r in range(NR)]
            layer_norm_batch([(h[r][:rw], rw, hn[r][:rw])
                              for r, (ro, rw) in enumerate(ROWS)], "phb")
            for r, (ro, rw) in enumerate(ROWS):
                psT = p2_tile()[:, :rw]
                T(psT, hn[r][:rw], ident[:rw, :rw])
                nc.vector.tensor_copy(hnT[r][:, :rw], psT)

                k1t = rtile([128, D], f32, "k1t")
                k2t = rtile([128, D], f32, "k2t")
                vt = rtile([128, D], f32, "vt")
                for dst, W, bB in ((k1t, Wk1, bk1_B), (k2t, Wk2, bk2_B),
                                   (vt, Wv, bv_B)):
                    psp = p1_tile()[:rw, :D]
                    nc.tensor.matmul(psp, hnT[r][:, :rw], W[:])
                    nc.vector.tensor_add(dst[:rw], psp, bB[:rw])
                # two disjoint column groups in one bank: safe on HW
                # (per-element has_written), only the sim's zero-region
                # check would object
                nc.tensor.matmul(gp[:, :D], k1t[:rw], vt[:rw],
                                 start=(r == 0), stop=(r == NR - 1),
                                 skip_group_check=True)
                nc.tensor.matmul(gp[:, D:2 * D], k2t[:rw], vt[:rw],
                                 start=(r == 0), stop=(r == NR - 1),
                                 skip_group_check=True)

        def emit_watt():
            # gram rides in the last AR group (bf16); cast back to f32 on chip
            kv16 = wtile([128, 2 * D], bf16, "kv16")
            nc.scalar.dma_start(out=kv16[:],
                                in_=p1_out[3][:, 1920:1920 + 2 * D])
            kv = wtile([128, 2 * D], f32, "kv")
            nc.vector.tensor_copy(kv[:], kv16[:])
            psq1T = mm_tile()[:, :D]
            T(psq1T, Wq1[:], ident[:])
            Wq1T = wtile([128, D], f32, "Wq1T")
            nc.vector.tensor_copy(Wq1T[:], psq1T)
            psq2T = mm_tile()[:, :D]
            T(psq2T, Wq2[:], ident[:])
            Wq2T = wtile([128, D], f32, "Wq2T")
            nc.vector.tensor_copy(Wq2T[:], psq2T)

            ps_w1e = mm_tile()[:, :D]
            nc.tensor.matmul(ps_w1e, Wq1T[:], kv[:, :D])
            Watt = wtile([128, D], f32, "Watt")
            nc.vector.tensor_copy(Watt[:], ps_w1e)
            ps_w2e = mm_tile()[:, :D]
            nc.tensor.matmul(ps_w2e, Wq2T[:], kv[:, D:])
            tmp2 = wtile([128, D], f32, "tmp2")
            nc.vector.tensor_scalar(tmp2[:], ps_w2e, scalar1=neglam_c,
                                    scalar2=None, op0=ALU.mult)
            nc.vector.tensor_add(Watt[:], Watt[:], tmp2[:])

            ps_b1 = mm_tile()[:1, :D]
            nc.tensor.matmul(ps_b1, bq1_c[:], kv[:, :D])
            batt_r = wtile([1, D], f32, "batt_r")
            nc.vector.tensor_copy(batt_r[:], ps_b1)
            ps_b2 = mm_tile()[:1, :D]
            nc.tensor.matmul(ps_b2, bq2_c[:], kv[:, D:])
            tmpb = wtile([1, D], f32, "tmpb")
            nc.vector.tensor_scalar(tmpb[:], ps_b2, scalar1=neglam_c[:1],
                                    scalar2=None, op0=ALU.mult)
            nc.vector.tensor_add(batt_r[:], batt_r[:], tmpb[:])
            batt_B = bcast_row("batt_B", batt_r)
            return Watt, batt_B

        # ================= spectral pipeline =================
        # the last AR group additionally carries the 2*D gram columns (bf16)
        ar_widths = [2048, 2048, 4096, 1920 + 2 * D]
        p1_in, p1_out = [], []
        for g, w in enumerate(ar_widths):
            p1_in.append(dram.tile([128, w], bf16, tag=f"p1in{g}",
                                   name=f"p1in{g}"))
            p1_out.append(dram.tile([128, w], bf16, tag=f"p1out{g}",
                                    name=f"p1out{g}", addr_space=shared_space))
        utxs_t = {}
        ut_tiles = {}

        def group_of(co):
            for g, (go, gw) in enumerate(AR_GROUPS):
                if go <= co < go + gw:
                    return g, go
            raise AssertionError

        def emit_chunk_pass1(c):
            co, cw = chunks[c]
            g, go = group_of(co)
            if g not in utxs_t:
                utxs_t[g] = utxst.tile([128, 4096 + 2 * D], bf16, tag="utxs",
                                       name=f"utxs{g}")
            # prefetch pass-2 uT tiles on the scalar HWDGE ring (parallel to
            # the sync ring carrying the u16 stream)
            for gi in range(16 * c, min(16 * (c + 1), NSUB)):
                utt = utstream.tile([128, NLOC], bf16, tag="ut",
                                    name=f"ut{gi}")
                nc.scalar.dma_start(out=utt[:],
                                    in_=t_ut16[gi * 128:(gi + 1) * 128, :])
                ut_tiles[gi] = utt
            cbs = _splits(cw, 512)
            ps1 = [p1_tile()[:, :bw] for bo, bw in cbs]
            for r, (ro, rw) in enumerate(ROWS):
                ut = ustream.tile([128, 2048], bf16, tag="u",
                                  name=f"u{c}_{r}")[:rw, :cw]
                nc.sync.dma_start(out=ut, in_=t_u16[ro:ro + rw, co:co + cw])
                for b, (bo, bw) in enumerate(cbs):
                    nc.tensor.matmul(ps1[b], h16[r][:rw], ut[:, bo:bo + bw],
                                     start=(r == 0), stop=(r == NR - 1))
            lo = co - go
            for b, (bo, bw) in enumerate(cbs):
                nc.vector.tensor_copy(utxs_t[g][:, lo + bo:lo + bo + bw],
                                      ps1[b])

        def emit_ar(g):
            w = ar_widths[g]
            if g == 3:
                # append gram (cast to bf16) to this group's payload
                nc.vector.tensor_copy(utxs_t[3][:, 1920:1920 + 2 * D],
                                      gram_ps[0][:, :2 * D])
            nc.gpsimd.dma_start(out=p1_in[g][:], in_=utxs_t[g][:, :w])
            nc.gpsimd.collective_compute(
                "AllReduce", ALU.add, replica_groups=rg,
                ins=[p1_in[g].opt()], outs=[p1_out[g].opt()])

        ps2_acc = []   # persistent accumulators for henc^T, one per i-block

        def emit_pass2_group(g):
            go, gw = AR_GROUPS[g]
            nsub_g = gw // 128
            if not ps2_acc:
                for io, iw in IBLK:
                    ps2_acc.append(p2_tile()[:, :iw])
            # straight readback of the AR result on the (by now idle) sync ring
            utxr = utxrd.tile([128, 4096], bf16, tag="utxr",
                              name=f"utxr{g}")[:, :gw]
            nc.sync.dma_start(out=utxr, in_=p1_out[g][:, :gw])
            # batches of 4 subtiles: 4 PE transposes into one PSUM tile, then
            # 4 drain copies with the new_e scale fused (per-partition k)
            for b0 in range(0, nsub_g, 4):
                bn = min(4, nsub_g - b0)
                pz = ps_mm.tile([128, 512], bf16, tag="mm",
                                name=f"pz_{nc.next_id()}")
                for b in range(bn):
                    t = b0 + b
                    T(pz[:, b * 128:(b + 1) * 128],
                      utxr[:, t * 128:(t + 1) * 128], identb[:])
                zq = zpool.tile([128, 512], bf16, tag="z",
                                name=f"zq_{nc.next_id()}")
                for b in range(bn):
                    gi = go // 128 + b0 + b
                    nc.vector.tensor_scalar(zq[:, b * 128:(b + 1) * 128],
                                            pz[:, b * 128:(b + 1) * 128],
                                            scalar1=ne[:, gi:gi + 1],
                                            scalar2=None, op0=ALU.mult)
                for b in range(bn):
                    gi = go // 128 + b0 + b
                    first = (gi == 0)
                    last = (gi == NSUB - 1)
                    for ib, (io, iw) in enumerate(IBLK):
                        nc.tensor.matmul(ps2_acc[ib],
                                         zq[:, b * 128:(b + 1) * 128],
                                         ut_tiles[gi][:, io:io + iw],
                                         start=first, stop=last)

        # ========== pipeline ==========
        # pass-2 slices trail their AllReduce by >=2 chunk times so the
        # tensor queue never stalls on a collective
        emit_chunk_pass1(0)
        emit_ar(0)
        emit_phase_b()
        emit_chunk_pass1(1)
        emit_ar(1)
        emit_chunk_pass1(2)
        emit_chunk_pass1(3)
        emit_ar(2)
        emit_pass2_group(0)
        emit_chunk_pass1(4)
        emit_ar(3)
        emit_pass2_group(1)
        emit_pass2_group(2)

        # == attention (gram AR completed during early chunks) ==
        Watt, batt_B = emit_watt()
        ha = [wtile([128, D], f32, f"ha{r}") for r in range(NR)]
        s_sbs = []
        for r, (ro, rw) in enumerate(ROWS):
            pss = p1_tile()[:rw, :D]
            nc.tensor.matmul(pss, hnT[r][:, :rw], Watt[:])
            s_sb = wtile([128, D], f32, f"s_sb{r}")
            nc.vector.tensor_add(s_sb[:rw], pss, batt_B[:rw])
            s_sbs.append(s_sb)
        layer_norm_batch([(s_sbs[r][:rw], rw, s_sbs[r][:rw])
                          for r, (ro, rw) in enumerate(ROWS)], "attn")
        for r, (ro, rw) in enumerate(ROWS):
            psT = mm_tile()[:, :rw]
            T(psT, s_sbs[r][:rw], ident[:rw, :rw])
            sT = rtile([128, 128], f32, "sT")
            nc.vector.tensor_copy(sT[:, :rw], psT)
            psa = p1_tile()[:rw, :D]
            nc.tensor.matmul(psa, sT[:, :rw], Wo[:])
            att = rtile([128, D], f32, "att")
            nc.vector.tensor_add(att[:rw], psa, bo_B[:rw])
            nc.vector.tensor_add(ha[r][:rw], h[r][:rw], att[:rw])

        emit_pass2_group(3)

        # ================= residual + FFN =================
        hencT = wtile([128, NLOC], f32, "hencT")
        for ib, (io, iw) in enumerate(IBLK):
            nc.vector.tensor_copy(hencT[:, io:io + iw], ps2_acc[ib])

        mh = [wtile([128, D], f32, f"mh{r}") for r in range(NR)]
        gl = [wtile([128, D], f32, f"gl{r}") for r in range(NR)]
        fh = [wtile([128, D], f32, f"fh{r}") for r in range(NR)]
        for r, (ro, rw) in enumerate(ROWS):
            psb = p2_tile()[:rw, :D]
            T(psb, hencT[:, ro:ro + rw], ident[:])
            nc.vector.tensor_add(mh[r][:rw], ha[r][:rw], psb)
        layer_norm_batch([(mh[r][:rw], rw, fh[r][:rw])
                          for r, (ro, rw) in enumerate(ROWS)], "ffn")
        for r, (ro, rw) in enumerate(ROWS):
            psT = p2_tile()[:, :rw]
            T(psT, fh[r][:rw], ident[:rw, :rw])
            fT = rtile([128, 128], f32, "fT")
            nc.vector.tensor_copy(fT[:, :rw], psT)
            psg = p1_tile()[:rw, :D]
            nc.tensor.matmul(psg, fT[:, :rw], W1p[:])
            nc.vector.tensor_add(gl[r][:rw], psg, b1p_B[:rw])
        for r, (ro, rw) in enumerate(ROWS):
            if sim_gelu:
                # tanh-approx gelu (CoreSim lacks Gelu); HW build uses AF.Gelu
                x3 = rtile([128, D], f32, "x3")
                nc.vector.tensor_mul(x3[:rw], gl[r][:rw], gl[r][:rw])
                nc.vector.tensor_mul(x3[:rw], x3[:rw], gl[r][:rw])
                nc.vector.tensor_scalar(x3[:rw], x3[:rw], scalar1=0.044715,
                                        scalar2=None, op0=ALU.mult)
                nc.vector.tensor_add(x3[:rw], x3[:rw], gl[r][:rw])
                nc.scalar.activation(x3[:rw], x3[:rw], AF.Tanh,
                                     scale=math.sqrt(2.0 / math.pi))
                nc.vector.tensor_scalar(x3[:rw], x3[:rw], scalar1=1.0,
                                        scalar2=0.5, op0=ALU.add, op1=ALU.mult)
                nc.vector.tensor_mul(gl[r][:rw], gl[r][:rw], x3[:rw])
            else:
                nc.scalar.activation(gl[r][:rw], gl[r][:rw], AF.Gelu)
        for r, (ro, rw) in enumerate(ROWS):
            psT2 = p2_tile()[:, :rw]
            T(psT2, gl[r][:rw], ident[:rw, :rw])
            gT = rtile([128, 128], f32, "gT")
            nc.vector.tensor_copy(gT[:, :rw], psT2)
            pso2 = p1_tile()[:rw, :D]
            nc.tensor.matmul(pso2, gT[:, :rw], f2w[:])
            outp = rtile([128, D], f32, "outp")
            nc.vector.tensor_add(outp[:rw], pso2, mh[r][:rw])
            nc.vector.tensor_add(outp[:rw], outp[:rw], f2b_B[:rw])
            nc.gpsimd.dma_start(out=t_out[ro:ro + rw, :], in_=outp[:rw])

    nc.compile()
    return nc


# ==================== host-side entry point ====================

_CACHED = {}


def _get_nc(N=N_FULL, NF=NF_FULL, CORES=CORES_FULL):
    key = (N, NF, CORES)
    if key not in _CACHED:
        _CACHED[key] = build_kernel(N, NF, CORES)
    return _CACHED[key]


def make_in_maps(inputs, N, CORES):
    import ml_dtypes

    bf16 = ml_dtypes.bfloat16
    NLOC = N // CORES
    full = {}
    for k, v in inputs.items():
        if k in ("u", "x"):
            continue
        full[k] = np.ascontiguousarray(np.asarray(v, dtype=np.float32))
    u = np.asarray(inputs["u"], dtype=np.float32)
    x = np.asarray(inputs["x"], dtype=np.float32)
    in_maps = []
    for c in range(CORES):
        rows = slice(c * NLOC, (c + 1) * NLOC)
        u_c = u[rows]
        u16 = np.zeros((NLOC, N_PAD), dtype=bf16)
        u16[:, :N] = u_c.astype(bf16)
        ut16 = np.zeros((N_PAD, NLOC), dtype=bf16)
        ut16[:N, :] = u_c.T.astype(bf16)
        xT = x[rows].T.astype(bf16)
        m = dict(full)
        m["u16"] = u16
        m["ut16"] = ut16
        m["xT"] = xT
        in_maps.append(m)
    return in_maps


def kernel(**inputs):
    from concourse import bass_utils

    nc = _get_nc()
    in_maps = make_in_maps(inputs, N_FULL, CORES_FULL)
    res = bass_utils.run_bass_kernel_spmd(nc, in_maps,
                                          core_ids=list(range(CORES_FULL)))
    out = np.concatenate([res.results[c]["out"] for c in range(CORES_FULL)],
                         axis=0)
    return out.astype(np.float32)


if __name__ == "__main__":
    build_kernel()
    print("build ok")
